# revision 2
# baseline (speedup 1.0000x reference)
"""Trainium2 Bass kernel for a dense transformer block (pre-LN MHA + MLP).

v2: fp8 e4m3 DoubleRow matmuls for the heavy GEMMs.

Sharding: pure data parallel - batch (8) maps 1:1 onto the 8 NeuronCores.

Quantization scheme (validated vs reference on CPU, rel_fro ~1.2e-2):
  - hT (LN1 out), v, probs (exp out), oT: single fp8, DR matmuls.
  - w_qkv, w_proj: single fp8 (x256 host scale).
  - fc1: 3-pass (h2 hi+lo fp8 x w1 hi+lo fp8; lo*lo term dropped).
  - fc2: w2 hi+lo fp8, a1 single fp8.
  - q/k stored fp16; scores matmuls fp16 (a DR head-dim pack would need
    cross-partition remaps the qkv matmul cannot produce).
  - softmax: probs = exp(s*scale + ln8) unnormalized fp8; S via one-hot
    DR matmuls into a [16, 512] psum per head pair; rb = sel @ fp16(1/S)
    broadcast by the PE; o = pav * rb.
  - DR matmuls must write psum base partition 0 (ISA): odd heads' AV
    runs non-DR fp8 into rows 64:128 of the pair psum.

Self-contained: hardcodes all shapes from the problem spec.
"""

from contextlib import ExitStack

import numpy as np
import ml_dtypes

import concourse.bass as bass
import concourse.tile as tile
from concourse import bacc, mybir
from concourse.bass import ts
from concourse.bass_utils import run_bass_kernel_spmd
from concourse.masks import make_identity

F32 = mybir.dt.float32
F16 = mybir.dt.float16
F8 = mybir.dt.float8e4
AF = mybir.ActivationFunctionType
ALU = mybir.AluOpType
DR = mybir.MatmulPerfMode.DoubleRow
E4 = ml_dtypes.float8_e4m3

P = 128          # partitions
N = 1024         # tokens per core
D = 1024         # model dim
KC = D // P      # 8 contraction chunks of 128
HEADS = 16
HD = 64          # head dim
HID = 4096
EPS = 1e-6
MT = N // P      # 8 token tiles of 128
SCALE = HD ** -0.5
WS = 256.0       # host weight scale
IWS = 1.0 / WS
LN8 = float(np.log(8.0))


def build_block(ln1_triv, ln2_triv, qk_triv, fc1b_triv, apply_c1, apply_bfc2):
    nc = bacc.Bacc("TRN2", target_bir_lowering=False, debug=False, num_devices=8)

    x_d = nc.dram_tensor("x", [N, D], F32, kind="ExternalInput")
    wqkv_d = nc.dram_tensor("w_qkv", [D, 3 * D], F8, kind="ExternalInput")
    wproj_d = nc.dram_tensor("w_proj", [D, D], F8, kind="ExternalInput")
    w1hi_d = nc.dram_tensor("w1hi", [D, HID], F8, kind="ExternalInput")
    w1lo_d = nc.dram_tensor("w1lo", [D, HID], F8, kind="ExternalInput")
    w2hi_d = nc.dram_tensor("w2hi", [HID, D], F8, kind="ExternalInput")
    bqkv_d = nc.dram_tensor("b_qkv", [3 * D], F32, kind="ExternalInput")
    bfc1_d = nc.dram_tensor("b_fc1", [HID], F32, kind="ExternalInput")
    ln1s_d = nc.dram_tensor("ln1_scale", [D], F32, kind="ExternalInput")
    ln1b_d = nc.dram_tensor("ln1_bias", [D], F32, kind="ExternalInput")
    ln2s_d = nc.dram_tensor("ln2_scale", [D], F32, kind="ExternalInput")
    ln2b_d = nc.dram_tensor("ln2_bias", [D], F32, kind="ExternalInput")
    sel_d = nc.dram_tensor("selc", [16, 2, HD], F16, kind="ExternalInput")
    ohe_d = nc.dram_tensor("ohe", [P, 2, 16], F8, kind="ExternalInput")
    oho_d = nc.dram_tensor("oho", [P, 2, 16], F8, kind="ExternalInput")
    c1_d = nc.dram_tensor("c1", [D], F32, kind="ExternalInput") if apply_c1 else None
    bfc2_d = (
        nc.dram_tensor("b_fc2c", [D], F32, kind="ExternalInput") if apply_bfc2 else None
    )
    y_d = nc.dram_tensor("y", [N, D], F16, kind="ExternalOutput")

    # [(kc p), n] -> [p, kc, n] views for weight loads
    wqkv_v = wqkv_d.ap().rearrange("(kc p) n -> p kc n", p=P)
    wproj_v = wproj_d.ap().rearrange("(kc p) n -> p kc n", p=P)
    w1hi_v = w1hi_d.ap().rearrange("(kc p) n -> p kc n", p=P)
    w1lo_v = w1lo_d.ap().rearrange("(kc p) n -> p kc n", p=P)
    w2hi_v = w2hi_d.ap().rearrange("(kc p) n -> p kc n", p=P)

    with tile.TileContext(nc) as tc, ExitStack() as ctx:
        ep = ctx.enter_context
        constp = ep(tc.tile_pool(name="const", bufs=1))
        xload = ep(tc.tile_pool(name="xload", bufs=2))
        htmpp = ep(tc.tile_pool(name="htmp", bufs=2))
        hTp = ep(tc.tile_pool(name="hT", bufs=1))
        h2hip = ep(tc.tile_pool(name="h2hi", bufs=1))
        h2lop = ep(tc.tile_pool(name="h2lo", bufs=1))
        qTp = ep(tc.tile_pool(name="qT", bufs=1))
        kTp = ep(tc.tile_pool(name="kT", bufs=1))
        vp = ep(tc.tile_pool(name="vv", bufs=1))
        oTp = ep(tc.tile_pool(name="oT", bufs=1))
        x1p = ep(tc.tile_pool(name="x1", bufs=1))
        probsp = ep(tc.tile_pool(name="probs", bufs=4))
        wp = ep(tc.tile_pool(name="w", bufs=4))
        wprojp = ep(tc.tile_pool(name="wproj", bufs=2))
        w2pool = ep(tc.tile_pool(name="w2", bufs=2))
        statsp = ep(tc.tile_pool(name="stats", bufs=4))
        srowp = ep(tc.tile_pool(name="srow", bufs=2))
        # PSUM: big [128,1024]x2 (4 banks) + pav [128,512]x2 (2) + aux
        # [128,512]x2 (2) = 8 banks
        bigp = ep(tc.tile_pool(name="big", bufs=2, space="PSUM"))
        pavp = ep(tc.tile_pool(name="pav", bufs=2, space="PSUM"))
        auxp = ep(tc.tile_pool(name="aux", bufs=2, space="PSUM"))

        # ---- first x tile load goes out before anything else ----
        x_t0 = xload.tile([P, D], F32, tag="x_t")
        nc.sync.dma_start(x_t0[:, 0:512], x_d.ap()[ts(0, P), 0:512])
        nc.sync.dma_start(x_t0[:, 512:1024], x_d.ap()[ts(0, P), 512:1024])

        # ---- constants (gpsimd queue; keeps sync queue on x) ----
        ident = constp.tile([P, P], F16)
        make_identity(nc, ident[:])
        eps_t = constp.tile([P, 1], F32)
        nc.vector.memset(eps_t[:], EPS)
        ln8_t = constp.tile([P, 1], F32)
        nc.vector.memset(ln8_t[:], LN8)
        sel_t = constp.tile([16, 2, HD], F16)
        nc.gpsimd.dma_start(sel_t[:], sel_d.ap())
        ohe_t = constp.tile([P, 2, 16], F8)
        nc.gpsimd.dma_start(ohe_t[:], ohe_d.ap())
        oho_t = constp.tile([P, 2, 16], F8)
        nc.gpsimd.dma_start(oho_t[:], oho_d.ap())
        ln1s = constp.tile([P, KC], F32)
        nc.gpsimd.dma_start(ln1s[:], ln1s_d.ap().rearrange("(k p) -> p k", p=P))
        ln1b = constp.tile([P, KC], F32)
        nc.gpsimd.dma_start(ln1b[:], ln1b_d.ap().rearrange("(k p) -> p k", p=P))
        ln2s = constp.tile([P, KC], F32)
        nc.gpsimd.dma_start(ln2s[:], ln2s_d.ap().rearrange("(k p) -> p k", p=P))
        ln2b = constp.tile([P, KC], F32)
        nc.gpsimd.dma_start(ln2b[:], ln2b_d.ap().rearrange("(k p) -> p k", p=P))
        bqk = constp.tile([P, 16], F32)
        bqkv_v = bqkv_d.ap().rearrange("(m p) -> p m", p=P)
        nc.gpsimd.dma_start(bqk[:], bqkv_v[:, 0:16])
        bfc1 = constp.tile([P, HID // P], F32)
        nc.gpsimd.dma_start(bfc1[:], bfc1_d.ap().rearrange("(m p) -> p m", p=P))
        if apply_c1:
            c1row = constp.tile([P, D], F32)
            src = c1_d.ap()
            nc.gpsimd.dma_start(
                c1row[:],
                bass.AP(tensor=src.tensor, offset=src.offset, ap=[[0, P], [1, D]]),
            )
        if apply_bfc2:
            b2row = constp.tile([P, D], F32)
            src = bfc2_d.ap()
            nc.gpsimd.dma_start(
                b2row[:],
                bass.AP(tensor=src.tensor, offset=src.offset, ap=[[0, P], [1, D]]),
            )

        def layer_norm_to(src_ap, out_hi, out_lo, s_cols, b_cols, mt, trivial):
            """LN over free dim of src [128, 1024]; write transposed fp8 into
            out_hi[:, kc, mt*128:...] (+ optional fp8 lo residual)."""
            st = statsp.tile([P, 2, 6], F32, tag="st")
            xr = src_ap.rearrange("p (a b) -> p a b", b=512)
            nc.vector.bn_stats(st[:, 0, :], xr[:, 0, :])
            nc.vector.bn_stats(st[:, 1, :], xr[:, 1, :])
            mv = statsp.tile([P, 2], F32, tag="mv")
            nc.vector.bn_aggr(mv[:], st[:])
            rstd = statsp.tile([P, 1], F32, tag="rstd")
            nc.scalar.activation(rstd[:], mv[:, 1:2], AF.Sqrt, bias=eps_t[:])
            nc.vector.reciprocal(rstd[:], rstd[:])
            h = htmpp.tile([P, D], F16, tag="h")
            nc.vector.tensor_scalar(
                out=h[:, 0:512], in0=src_ap[:, 0:512], scalar1=mv[:, 0:1],
                scalar2=rstd[:], op0=ALU.subtract, op1=ALU.mult,
            )
            nc.gpsimd.tensor_scalar(
                out=h[:, 512:1024], in0=src_ap[:, 512:1024], scalar1=mv[:, 0:1],
                scalar2=rstd[:], op0=ALU.subtract, op1=ALU.mult,
            )
            for kg in range(2):  # transpose batches of 4 kc blocks
                pt_t = auxp.tile([P, 512], F16, tag="aux")
                for kl in range(4):
                    kc = kg * 4 + kl
                    nc.tensor.transpose(pt_t[:, ts(kl, P)], h[:, ts(kc, P)], ident[:])
                dst = out_hi[:, kg * 4 : kg * 4 + 4, ts(mt, P)]
                src4 = pt_t[:].rearrange("p (k c) -> p k c", c=P)
                if trivial:
                    if out_lo is None:
                        # LN1: split copies DVE/ACT
                        if kg == 1:
                            nc.scalar.copy(dst, src4)
                        else:
                            nc.vector.tensor_copy(dst, src4)
                    else:
                        # LN2: ACT writes hi fp8 + full fp16; Pool (sbuf-only)
                        # computes lo so the DVE stays free for proj/stats
                        h216 = htmpp.tile([P, 512], F16, tag="h216")
                        nc.scalar.copy(dst, src4)
                        nc.scalar.copy(h216[:], pt_t[:])
                        nc.gpsimd.tensor_tensor(
                            out=out_lo[:, kg * 4 : kg * 4 + 4, ts(mt, P)],
                            in0=h216[:].rearrange("p (k c) -> p k c", c=P),
                            in1=dst, op=ALU.subtract,
                        )
                else:
                    for kl in range(4):
                        kc = kg * 4 + kl
                        nc.vector.tensor_scalar(
                            out=out_hi[:, kc, ts(mt, P)], in0=pt_t[:, ts(kl, P)],
                            scalar1=s_cols[:, kc : kc + 1],
                            scalar2=b_cols[:, kc : kc + 1],
                            op0=ALU.mult, op1=ALU.add,
                        )
                        if out_lo is not None:
                            tmp = htmpp.tile([P, P], F16, tag="lntmp")
                            nc.vector.tensor_scalar(
                                out=tmp[:], in0=pt_t[:, ts(kl, P)],
                                scalar1=s_cols[:, kc : kc + 1],
                                scalar2=b_cols[:, kc : kc + 1],
                                op0=ALU.mult, op1=ALU.add,
                            )
                            nc.gpsimd.tensor_tensor(
                                out=out_lo[:, kc, ts(mt, P)], in0=tmp[:],
                                in1=out_hi[:, kc, ts(mt, P)], op=ALU.subtract,
                            )

        hT = hTp.tile([P, KC, N], F8, tag="hT")

        # ---- phase 1: LN1 + transpose -> hT fp8 ----
        for mt in range(MT):
            if mt == 0:
                x_t = x_t0
            else:
                x_t = xload.tile([P, D], F32, tag="x_t")
                nc.sync.dma_start(x_t[:, 0:512], x_d.ap()[ts(mt, P), 0:512])
                nc.sync.dma_start(x_t[:, 512:1024], x_d.ap()[ts(mt, P), 512:1024])
            layer_norm_to(x_t[:], hT, None, ln1s, ln1b, mt, ln1_triv)

        # ---- phase 2: qkv (fp8 DR) ----
        qT = qTp.tile([P, KC, N], F16, tag="qT")
        kT = kTp.tile([P, KC, N], F16, tag="kT")
        v_sb = vp.tile([P, MT, HEADS * HD], F8, tag="vv")

        def wpiece(view, n0, pool=None, tag="w"):
            pool = pool or wp
            t = pool.tile([P, KC, 512], F8, tag=tag)
            nc.sync.dma_start(t[:], view[:, :, n0 : n0 + 512])
            return t

        for half in range(2):  # 0: q (cols 0:1024), 1: k (cols 1024:2048)
            pieces = [wpiece(wqkv_v, half * 1024), wpiece(wqkv_v, half * 1024 + 512)]
            dst_t = qT if half == 0 else kT
            for mc in range(8):
                piece = pieces[mc // 4]
                mc_l = mc % 4
                ps = bigp.tile([P, N], F32, tag="big")
                for nt in range(2):
                    for kk in range(4):
                        nc.tensor.matmul(
                            ps[:, ts(nt, 512)],
                            piece[:, 2 * kk : 2 * kk + 2, ts(mc_l, P)],
                            hT[:, 2 * kk : 2 * kk + 2, ts(nt, 512)],
                            start=(kk == 0), stop=(kk == 3),
                            perf_mode=DR, skip_group_check=True,
                        )
                dst = dst_t[:, mc, :]
                if qk_triv:
                    if mc % 2 == 0:
                        nc.vector.tensor_scalar(
                            out=dst, in0=ps[:], scalar1=IWS, scalar2=None,
                            op0=ALU.mult,
                        )
                    else:
                        nc.scalar.activation(dst, ps[:], AF.Copy, scale=IWS)
                else:
                    mcg = half * 8 + mc
                    nc.vector.tensor_scalar(
                        out=dst, in0=ps[:], scalar1=IWS,
                        scalar2=bqk[:, mcg : mcg + 1], op0=ALU.mult, op1=ALU.add,
                    )

        v_pieces = [wpiece(wqkv_v, 2048), wpiece(wqkv_v, 2560)]
        for mt in range(MT):
            ps = bigp.tile([P, N], F32, tag="big")
            for nv in range(2):
                for kk in range(4):
                    nc.tensor.matmul(
                        ps[:, ts(nv, 512)],
                        hT[:, 2 * kk : 2 * kk + 2, ts(mt, P)],
                        v_pieces[nv][:, 2 * kk : 2 * kk + 2, :],
                        start=(kk == 0), stop=(kk == 3),
                        perf_mode=DR, skip_group_check=True,
                    )
            # v bias folds into c1 after proj (normalized probs sum to 1)
            if mt % 2 == 0:
                nc.vector.tensor_scalar(
                    out=v_sb[:, mt, :], in0=ps[:], scalar1=IWS, scalar2=None,
                    op0=ALU.mult,
                )
            else:
                nc.scalar.activation(v_sb[:, mt, :], ps[:], AF.Copy, scale=IWS)

        # w_proj load early (streams behind attention)
        proj_pieces = [wpiece(wproj_v, 0, wprojp, "wproj"),
                       wpiece(wproj_v, 512, wprojp, "wproj")]

        # ---- phase 3: attention, two query halves ----
        oT = oTp.tile([P, KC, N], F8, tag="oT")

        def scores_exp(h, probs_h, kk, nq):
            """scores psum [128, 2, 512] for mk=2kk,2kk+1 -> exp -> probs fp8."""
            mc = h // 2
            pr = (h % 2) * HD
            sc = bigp.tile([P, 2, 512], F32, tag="big")
            for j in range(2):
                nc.tensor.matmul(
                    sc[:, j, :],
                    kT[pr : pr + HD, mc, ts(2 * kk + j, P)],
                    qT[pr : pr + HD, mc, ts(nq, 512)],
                    start=True, stop=True, skip_group_check=True,
                )
            nc.scalar.activation(
                probs_h[:, 2 * kk : 2 * kk + 2, :], sc[:], AF.Exp,
                bias=ln8_t[:], scale=SCALE,
            )

        for nq in range(2):
            probs_q = []

            def emit_head(h, nq=nq):
                probs_h = probsp.tile([P, MT, 512], F8, tag="probs")
                for kk in range(4):
                    scores_exp(h, probs_h, kk, nq)
                probs_q.append(probs_h)

            emit_head(0)
            emit_head(1)
            for m in range(8):  # head pairs (2m, 2m+1)
                h0, h1 = 2 * m, 2 * m + 1
                p_h0 = probs_q.pop(0)
                p_h1 = probs_q.pop(0)
                pav = pavp.tile([P, 512], F32, tag="pav")
                sps = auxp.tile([16, 512], F32, tag="aux")
                # even head: DR at base 0 (rows 0:64)
                for kk in range(4):
                    nc.tensor.matmul(
                        pav[0:HD, :],
                        v_sb[:, 2 * kk : 2 * kk + 2, h0 * HD : (h0 + 1) * HD],
                        p_h0[:, 2 * kk : 2 * kk + 2, :],
                        start=(kk == 0), stop=(kk == 3),
                        perf_mode=DR, skip_group_check=True,
                    )
                # S for even head (one-hot col 0)
                for kk in range(4):
                    nc.tensor.matmul(
                        sps[:], ohe_t[:],
                        p_h0[:, 2 * kk : 2 * kk + 2, :],
                        start=(kk == 0), stop=False,
                        perf_mode=DR, skip_group_check=True,
                    )
                if m < 7:
                    emit_head(2 * m + 2)
                # odd head: non-DR fp8 at base 64 (rows 64:128)
                for mk in range(MT):
                    nc.tensor.matmul(
                        pav[HD:P, :],
                        v_sb[:, mk, h1 * HD : (h1 + 1) * HD],
                        p_h1[:, mk, :],
                        start=(mk == 0), stop=(mk == MT - 1),
                        skip_group_check=True,
                    )
                # S for odd head (one-hot col 1) closes the pair group
                for kk in range(4):
                    nc.tensor.matmul(
                        sps[:], oho_t[:],
                        p_h1[:, 2 * kk : 2 * kk + 2, :],
                        start=False, stop=(kk == 3),
                        perf_mode=DR, skip_group_check=True,
                    )
                if m < 7:
                    emit_head(2 * m + 3)
                # drain S -> 1/S -> fp16 -> rb broadcast -> oT = pav * rb
                # (rows 2:15 of sps are zero; keep them out of the reciprocal
                # so no inf reaches the rb matmul)
                srow = srowp.tile([16, 512], F32, tag="srow")
                nc.vector.reciprocal(srow[0:2, :], sps[0:2, :])
                srow16 = srowp.tile([16, 512], F16, tag="srow16")
                nc.vector.tensor_copy(srow16[0:2, :], srow[0:2, :])
                rb = auxp.tile([P, 512], F32, tag="aux")
                nc.tensor.matmul(rb[:], sel_t[0:2, :, :], srow16[0:2, :],
                                 start=True, stop=True)
                rb16 = srowp.tile([P, 512], F16, tag="rb16")
                nc.vector.tensor_copy(rb16[:], rb[:])
                nc.vector.tensor_tensor(
                    out=oT[:, m, ts(nq, 512)], in0=pav[:], in1=rb16[:],
                    op=ALU.mult,
                )

        # ---- phase 4+5: proj + residual -> x1, LN2 fused per tile ----
        x1 = x1p.tile([P, MT, D], F16)
        h2hi = h2hip.tile([P, KC, N], F8, tag="h2hi")
        h2lo = h2lop.tile([P, KC, N], F8, tag="h2lo")
        for mt in range(MT):
            x_t = xload.tile([P, D], F32, tag="x_t")
            nc.sync.dma_start(x_t[:], x_d.ap()[ts(mt, P), :])
            ps = bigp.tile([P, N], F32, tag="big")
            for np_ in range(2):
                for kk in range(4):
                    nc.tensor.matmul(
                        ps[:, ts(np_, 512)],
                        oT[:, 2 * kk : 2 * kk + 2, ts(mt, P)],
                        proj_pieces[np_][:, 2 * kk : 2 * kk + 2, :],
                        start=(kk == 0), stop=(kk == 3),
                        perf_mode=DR, skip_group_check=True,
                    )
            nc.vector.scalar_tensor_tensor(
                x1[:, mt, :], ps[:], IWS, x_t[:], ALU.mult, ALU.add,
            )
            if apply_c1:
                nc.vector.tensor_add(x1[:, mt, :], x1[:, mt, :], c1row[:])
            layer_norm_to(x1[:, mt, :], h2hi, h2lo, ln2s, ln2b, mt, ln2_triv)

        # ---- phase 6: fc1 (3-pass fp8 DR) + gelu -> a1 fp8 ----
        # a1 [128, 32, 1024] fp8 reuses qT (chunks 0:16) + kT (16:32) slots
        a1a = qTp.tile([P, 16, N], F8, tag="qT")
        a1b = kTp.tile([P, 16, N], F8, tag="kT")

        for p8 in range(8):  # 512-wide hidden column pieces
            w1h_t = wpiece(w1hi_v, p8 * 512)
            w1l_t = wpiece(w1lo_v, p8 * 512)
            for nt in range(2):
                for mg in range(2):  # two mh chunks per psum
                    ps = bigp.tile([P, 2, 512], F32, tag="big")
                    for ml in range(2):
                        mh_l = mg * 2 + ml
                        for kk in range(4):
                            k2 = slice(2 * kk, 2 * kk + 2)
                            for wt, ht in ((w1h_t, h2hi), (w1l_t, h2hi),
                                           (w1h_t, h2lo)):
                                nc.tensor.matmul(
                                    ps[:, ml, :],
                                    wt[:, k2, ts(mh_l, P)],
                                    ht[:, k2, ts(nt, 512)],
                                    start=(kk == 0 and wt is w1h_t and ht is h2hi),
                                    stop=(kk == 3 and ht is h2lo),
                                    perf_mode=DR, skip_group_check=True,
                                )
                    mhg = p8 * 4 + mg * 2  # first of two mh chunks
                    a1_t = a1a if mhg < 16 else a1b
                    adst = a1_t[:, mhg % 16 : mhg % 16 + 2, ts(nt, 512)]
                    if fc1b_triv:
                        nc.scalar.activation(
                            adst, ps[:], AF.Gelu_apprx_tanh,
                            bias=bfc1[:, 0:1], scale=IWS,
                        )
                    else:
                        for ml in range(2):
                            nc.scalar.activation(
                                a1_t[:, mhg % 16 + ml, ts(nt, 512)],
                                ps[:, ml, :], AF.Gelu_apprx_tanh,
                                bias=bfc1[:, mhg + ml : mhg + ml + 1], scale=IWS,
                            )

        # ---- phase 7: fc2 (1-pass fp8 DR) + residual -> y ----
        # quarter-K phases: w2 quarter tiles [128, 8, 1024] fp8; x1
        # accumulates the first three quarters, last quarter writes y.
        # (w2 single fp8: measured rel_fro 1.67e-2 vs gate 2e-2.)
        for qk_ in range(4):
            w2h_t = w2pool.tile([P, KC, N], F8, tag="w2h")
            nc.sync.dma_start(w2h_t[:], w2hi_v[:, qk_ * 8 : qk_ * 8 + 8, :])
            a1_t = a1a if qk_ < 2 else a1b
            kbase = (qk_ % 2) * 8
            for mt in range(MT):
                ps = bigp.tile([P, N], F32, tag="big")
                for ncol in range(2):
                    for kk in range(4):
                        a2 = slice(kbase + 2 * kk, kbase + 2 * kk + 2)
                        k2 = slice(2 * kk, 2 * kk + 2)
                        nc.tensor.matmul(
                            ps[:, ts(ncol, 512)],
                            a1_t[:, a2, ts(mt, P)],
                            w2h_t[:, k2, ts(ncol, 512)],
                            start=(kk == 0), stop=(kk == 3),
                            perf_mode=DR, skip_group_check=True,
                        )
                if qk_ < 3:
                    nc.vector.scalar_tensor_tensor(
                        x1[:, mt, :], ps[:], IWS, x1[:, mt, :], ALU.mult, ALU.add,
                    )
                else:
                    y_sb = xload.tile([P, D], F16, tag="y_sb")
                    nc.vector.scalar_tensor_tensor(
                        y_sb[:], ps[:], IWS, x1[:, mt, :], ALU.mult, ALU.add,
                    )
                    if apply_bfc2:
                        nc.vector.tensor_add(y_sb[:], y_sb[:], b2row[:])
                    nc.sync.dma_start(y_d.ap()[ts(mt, P), :], y_sb[:])

    nc.compile()
    return nc


_cache = {}


def _get_nc(*key):
    if key not in _cache:
        _cache[key] = build_block(*key)
    return _cache[key]


def _host_consts():
    # rb matmul (non-DR): out[jm, n] = sum_k sel[k, j, m] * srow[k, n] with the
    # lhsT free index (j, m) flattened onto output partitions: partitions 0:64
    # (j=0, even head) read srow row 0; partitions 64:128 (j=1, odd) row 1.
    sel = np.zeros((16, 2, HD), np.float16)
    sel[0, 0, :] = 1.0
    sel[1, 1, :] = 1.0
    ohe = np.zeros((P, 2, 16), np.float32)
    ohe[:, :, 0] = 1.0
    oho = np.zeros((P, 2, 16), np.float32)
    oho[:, :, 1] = 1.0
    return sel, ohe.astype(E4), oho.astype(E4)


def kernel(
    x, w_qkv, b_qkv, w_proj, b_proj, ln1_scale, ln1_bias,
    ln2_scale, ln2_bias, w_fc1, b_fc1, w_fc2, b_fc2,
):
    x = np.asarray(x, np.float32)
    B = x.shape[0]
    b_qkv = np.asarray(b_qkv, np.float32)
    b_v = b_qkv[2 * D :]
    # exact fold: o includes +b_v after softmax-normalize (rows sum to 1),
    # so c1 = b_v @ w_proj + b_proj is a constant row added post-proj.
    c1 = b_v.astype(np.float64) @ np.asarray(w_proj, np.float64) + np.asarray(
        b_proj, np.float64
    )
    c1 = c1.astype(np.float32)
    bfc2 = np.asarray(b_fc2, np.float32)
    b_fc1 = np.asarray(b_fc1, np.float32)
    ln1_scale = np.asarray(ln1_scale, np.float32)
    ln1_bias = np.asarray(ln1_bias, np.float32)
    ln2_scale = np.asarray(ln2_scale, np.float32)
    ln2_bias = np.asarray(ln2_bias, np.float32)
    ln1_triv = bool(np.all(ln1_scale == 1) and np.all(ln1_bias == 0))
    ln2_triv = bool(np.all(ln2_scale == 1) and np.all(ln2_bias == 0))
    qk_triv = bool(np.all(b_qkv[: 2 * D] == 0))
    fc1b_triv = bool(np.all(b_fc1 == 0))
    apply_c1 = bool(np.any(c1 != 0))
    apply_bfc2 = bool(np.any(bfc2 != 0))

    nc = _get_nc(ln1_triv, ln2_triv, qk_triv, fc1b_triv, apply_c1, apply_bfc2)

    def q8(a):
        return np.ascontiguousarray(a).astype(E4)

    w_qkv8 = q8(np.asarray(w_qkv, np.float32) * WS)
    w_proj8 = q8(np.asarray(w_proj, np.float32) * WS)
    w1s = np.asarray(w_fc1, np.float32) * WS
    w1hi = q8(w1s)
    w1lo = q8(w1s - w1hi.astype(np.float32))
    w2s = np.asarray(w_fc2, np.float32) * WS
    w2hi = q8(w2s)
    sel, ohe, oho = _host_consts()

    base = {
        "w_qkv": w_qkv8,
        "w_proj": w_proj8,
        "w1hi": w1hi,
        "w1lo": w1lo,
        "w2hi": w2hi,
        "b_qkv": b_qkv,
        "b_fc1": b_fc1,
        "ln1_scale": ln1_scale,
        "ln1_bias": ln1_bias,
        "ln2_scale": ln2_scale,
        "ln2_bias": ln2_bias,
        "selc": sel,
        "ohe": ohe,
        "oho": oho,
    }
    if apply_c1:
        base["c1"] = c1
    if apply_bfc2:
        base["b_fc2c"] = bfc2

    in_maps = [dict(base, x=np.ascontiguousarray(x[i])) for i in range(B)]
    last_err = None
    for _attempt in range(3):
        try:
            res = run_bass_kernel_spmd(nc, in_maps, core_ids=list(range(B)))
            break
        except Exception as e:  # transient NRT/axon worker failures
            last_err = e
            import time as _time

            _time.sleep(2.0)
    else:
        raise last_err
    out = np.stack([res.results[i]["y"] for i in range(B)], axis=0)
    return np.ascontiguousarray(out.astype(np.float32))


# revision 3
# speedup vs baseline: 1.0280x; 1.0280x over previous
"""Trainium2 Bass kernel for a dense transformer block (pre-LN MHA + MLP).

v2: fp8 e4m3 DoubleRow matmuls for the heavy GEMMs.

Sharding: pure data parallel - batch (8) maps 1:1 onto the 8 NeuronCores.

Quantization scheme (validated vs reference on CPU, rel_fro ~1.2e-2):
  - hT (LN1 out), v, probs (exp out), oT: single fp8, DR matmuls.
  - w_qkv, w_proj: single fp8 (x256 host scale).
  - fc1: 3-pass (h2 hi+lo fp8 x w1 hi+lo fp8; lo*lo term dropped).
  - fc2: w2 hi+lo fp8, a1 single fp8.
  - q/k stored fp16; scores matmuls fp16 (a DR head-dim pack would need
    cross-partition remaps the qkv matmul cannot produce).
  - softmax: probs = exp(s*scale + ln8) unnormalized fp8; S via one-hot
    DR matmuls into a [16, 512] psum per head pair; rb = sel @ fp16(1/S)
    broadcast by the PE; o = pav * rb.
  - DR matmuls must write psum base partition 0 (ISA): odd heads' AV
    runs non-DR fp8 into rows 64:128 of the pair psum.

Self-contained: hardcodes all shapes from the problem spec.
"""

from contextlib import ExitStack

import numpy as np
import ml_dtypes

import concourse.bass as bass
import concourse.tile as tile
from concourse import bacc, mybir
from concourse.bass import ts
from concourse.bass_utils import run_bass_kernel_spmd
from concourse.masks import make_identity

F32 = mybir.dt.float32
F16 = mybir.dt.float16
F8 = mybir.dt.float8e4
AF = mybir.ActivationFunctionType
ALU = mybir.AluOpType
DR = mybir.MatmulPerfMode.DoubleRow
E4 = ml_dtypes.float8_e4m3

P = 128          # partitions
N = 1024         # tokens per core
D = 1024         # model dim
KC = D // P      # 8 contraction chunks of 128
HEADS = 16
HD = 64          # head dim
HID = 4096
EPS = 1e-6
MT = N // P      # 8 token tiles of 128
SCALE = HD ** -0.5
WS = 256.0       # host weight scale
IWS = 1.0 / WS
LN8 = float(np.log(8.0))


def build_block(ln1_triv, ln2_triv, qk_triv, fc1b_triv, apply_c1, apply_bfc2):
    nc = bacc.Bacc("TRN2", target_bir_lowering=False, debug=False, num_devices=8)

    x_d = nc.dram_tensor("x", [N, D], F32, kind="ExternalInput")
    wqkv_d = nc.dram_tensor("w_qkv", [D, 3 * D], F8, kind="ExternalInput")
    wproj_d = nc.dram_tensor("w_proj", [D, D], F8, kind="ExternalInput")
    w1hi_d = nc.dram_tensor("w1hi", [D, HID], F8, kind="ExternalInput")
    w1lo_d = nc.dram_tensor("w1lo", [D, HID], F8, kind="ExternalInput")
    w2hi_d = nc.dram_tensor("w2hi", [HID, D], F8, kind="ExternalInput")
    bqkv_d = nc.dram_tensor("b_qkv", [3 * D], F32, kind="ExternalInput")
    bfc1_d = nc.dram_tensor("b_fc1", [HID], F32, kind="ExternalInput")
    ln1s_d = nc.dram_tensor("ln1_scale", [D], F32, kind="ExternalInput")
    ln1b_d = nc.dram_tensor("ln1_bias", [D], F32, kind="ExternalInput")
    ln2s_d = nc.dram_tensor("ln2_scale", [D], F32, kind="ExternalInput")
    ln2b_d = nc.dram_tensor("ln2_bias", [D], F32, kind="ExternalInput")
    sel_d = nc.dram_tensor("selc", [16, 2, HD], F16, kind="ExternalInput")
    ohe_d = nc.dram_tensor("ohe", [P, 2, 16], F8, kind="ExternalInput")
    oho_d = nc.dram_tensor("oho", [P, 2, 16], F8, kind="ExternalInput")
    c1_d = nc.dram_tensor("c1", [D], F32, kind="ExternalInput") if apply_c1 else None
    bfc2_d = (
        nc.dram_tensor("b_fc2c", [D], F32, kind="ExternalInput") if apply_bfc2 else None
    )
    y_d = nc.dram_tensor("y", [N, D], F16, kind="ExternalOutput")

    # [(kc p), n] -> [p, kc, n] views for weight loads
    wqkv_v = wqkv_d.ap().rearrange("(kc p) n -> p kc n", p=P)
    wproj_v = wproj_d.ap().rearrange("(kc p) n -> p kc n", p=P)
    w1hi_v = w1hi_d.ap().rearrange("(kc p) n -> p kc n", p=P)
    w1lo_v = w1lo_d.ap().rearrange("(kc p) n -> p kc n", p=P)
    w2hi_v = w2hi_d.ap().rearrange("(kc p) n -> p kc n", p=P)

    with tile.TileContext(nc) as tc, ExitStack() as ctx:
        ep = ctx.enter_context
        constp = ep(tc.tile_pool(name="const", bufs=1))
        xload = ep(tc.tile_pool(name="xload", bufs=2))
        htmpp = ep(tc.tile_pool(name="htmp", bufs=2))
        hTp = ep(tc.tile_pool(name="hT", bufs=1))
        h2hip = ep(tc.tile_pool(name="h2hi", bufs=1))
        h2lop = ep(tc.tile_pool(name="h2lo", bufs=1))
        qTp = ep(tc.tile_pool(name="qT", bufs=1))
        kTp = ep(tc.tile_pool(name="kT", bufs=1))
        vp = ep(tc.tile_pool(name="vv", bufs=1))
        oTp = ep(tc.tile_pool(name="oT", bufs=1))
        x1p = ep(tc.tile_pool(name="x1", bufs=1))
        probsp = ep(tc.tile_pool(name="probs", bufs=4))
        wp = ep(tc.tile_pool(name="w", bufs=6))
        wprojp = ep(tc.tile_pool(name="wproj", bufs=2))
        w2pool = ep(tc.tile_pool(name="w2", bufs=1))
        statsp = ep(tc.tile_pool(name="stats", bufs=4))
        srowp = ep(tc.tile_pool(name="srow", bufs=2))
        # PSUM: big [128,1024]x2 (4 banks) + pav [128,512]x2 (2) + aux
        # [128,512]x2 (2) = 8 banks
        bigp = ep(tc.tile_pool(name="big", bufs=2, space="PSUM"))
        pavp = ep(tc.tile_pool(name="pav", bufs=2, space="PSUM"))
        auxp = ep(tc.tile_pool(name="aux", bufs=2, space="PSUM"))

        # ---- first x tile load goes out before anything else ----
        x_t0 = xload.tile([P, D], F32, tag="x_t")
        nc.sync.dma_start(x_t0[:, 0:512], x_d.ap()[ts(0, P), 0:512])
        nc.sync.dma_start(x_t0[:, 512:1024], x_d.ap()[ts(0, P), 512:1024])

        # ---- constants (gpsimd queue; keeps sync queue on x) ----
        ident = constp.tile([P, P], F16)
        make_identity(nc, ident[:])
        eps_t = constp.tile([P, 1], F32)
        nc.vector.memset(eps_t[:], EPS)
        ln8_t = constp.tile([P, 1], F32)
        nc.vector.memset(ln8_t[:], LN8)
        sel_t = constp.tile([16, 2, HD], F16)
        nc.gpsimd.dma_start(sel_t[:], sel_d.ap())
        ohe_t = constp.tile([P, 2, 16], F8)
        nc.gpsimd.dma_start(ohe_t[:], ohe_d.ap())
        oho_t = constp.tile([P, 2, 16], F8)
        nc.gpsimd.dma_start(oho_t[:], oho_d.ap())
        ln1s = constp.tile([P, KC], F32)
        nc.gpsimd.dma_start(ln1s[:], ln1s_d.ap().rearrange("(k p) -> p k", p=P))
        ln1b = constp.tile([P, KC], F32)
        nc.gpsimd.dma_start(ln1b[:], ln1b_d.ap().rearrange("(k p) -> p k", p=P))
        ln2s = constp.tile([P, KC], F32)
        nc.gpsimd.dma_start(ln2s[:], ln2s_d.ap().rearrange("(k p) -> p k", p=P))
        ln2b = constp.tile([P, KC], F32)
        nc.gpsimd.dma_start(ln2b[:], ln2b_d.ap().rearrange("(k p) -> p k", p=P))
        bqk = constp.tile([P, 16], F32)
        bqkv_v = bqkv_d.ap().rearrange("(m p) -> p m", p=P)
        nc.gpsimd.dma_start(bqk[:], bqkv_v[:, 0:16])
        bfc1 = constp.tile([P, HID // P], F32)
        nc.gpsimd.dma_start(bfc1[:], bfc1_d.ap().rearrange("(m p) -> p m", p=P))
        if apply_c1:
            c1row = constp.tile([P, D], F32)
            src = c1_d.ap()
            nc.gpsimd.dma_start(
                c1row[:],
                bass.AP(tensor=src.tensor, offset=src.offset, ap=[[0, P], [1, D]]),
            )
        if apply_bfc2:
            b2row = constp.tile([P, D], F32)
            src = bfc2_d.ap()
            nc.gpsimd.dma_start(
                b2row[:],
                bass.AP(tensor=src.tensor, offset=src.offset, ap=[[0, P], [1, D]]),
            )

        def layer_norm_to(src_ap, out_hi, out_lo, s_cols, b_cols, mt, trivial):
            """LN over free dim of src [128, 1024]; write transposed fp8 into
            out_hi[:, kc, mt*128:...] (+ optional fp8 lo residual)."""
            st = statsp.tile([P, 2, 6], F32, tag="st")
            xr = src_ap.rearrange("p (a b) -> p a b", b=512)
            nc.vector.bn_stats(st[:, 0, :], xr[:, 0, :])
            nc.vector.bn_stats(st[:, 1, :], xr[:, 1, :])
            mv = statsp.tile([P, 2], F32, tag="mv")
            nc.vector.bn_aggr(mv[:], st[:])
            rstd = statsp.tile([P, 1], F32, tag="rstd")
            nc.scalar.activation(rstd[:], mv[:, 1:2], AF.Sqrt, bias=eps_t[:])
            nc.vector.reciprocal(rstd[:], rstd[:])
            h = htmpp.tile([P, D], F16, tag="h")
            nc.vector.tensor_scalar(
                out=h[:, 0:512], in0=src_ap[:, 0:512], scalar1=mv[:, 0:1],
                scalar2=rstd[:], op0=ALU.subtract, op1=ALU.mult,
            )
            nc.gpsimd.tensor_scalar(
                out=h[:, 512:1024], in0=src_ap[:, 512:1024], scalar1=mv[:, 0:1],
                scalar2=rstd[:], op0=ALU.subtract, op1=ALU.mult,
            )
            for kg in range(2):  # transpose batches of 4 kc blocks
                pt_t = auxp.tile([P, 512], F16, tag="aux")
                for kl in range(4):
                    kc = kg * 4 + kl
                    nc.tensor.transpose(pt_t[:, ts(kl, P)], h[:, ts(kc, P)], ident[:])
                dst = out_hi[:, kg * 4 : kg * 4 + 4, ts(mt, P)]
                src4 = pt_t[:].rearrange("p (k c) -> p k c", c=P)
                if trivial:
                    if out_lo is None:
                        # LN1: split copies DVE/ACT
                        if kg == 1:
                            nc.scalar.copy(dst, src4)
                        else:
                            nc.vector.tensor_copy(dst, src4)
                    else:
                        # LN2: ACT writes hi fp8 + full fp16; Pool (sbuf-only)
                        # computes lo so the DVE stays free for proj/stats
                        h216 = htmpp.tile([P, 512], F16, tag="h216")
                        nc.scalar.copy(dst, src4)
                        nc.scalar.copy(h216[:], pt_t[:])
                        nc.gpsimd.tensor_tensor(
                            out=out_lo[:, kg * 4 : kg * 4 + 4, ts(mt, P)],
                            in0=h216[:].rearrange("p (k c) -> p k c", c=P),
                            in1=dst, op=ALU.subtract,
                        )
                else:
                    for kl in range(4):
                        kc = kg * 4 + kl
                        nc.vector.tensor_scalar(
                            out=out_hi[:, kc, ts(mt, P)], in0=pt_t[:, ts(kl, P)],
                            scalar1=s_cols[:, kc : kc + 1],
                            scalar2=b_cols[:, kc : kc + 1],
                            op0=ALU.mult, op1=ALU.add,
                        )
                        if out_lo is not None:
                            tmp = htmpp.tile([P, P], F16, tag="lntmp")
                            nc.vector.tensor_scalar(
                                out=tmp[:], in0=pt_t[:, ts(kl, P)],
                                scalar1=s_cols[:, kc : kc + 1],
                                scalar2=b_cols[:, kc : kc + 1],
                                op0=ALU.mult, op1=ALU.add,
                            )
                            nc.gpsimd.tensor_tensor(
                                out=out_lo[:, kc, ts(mt, P)], in0=tmp[:],
                                in1=out_hi[:, kc, ts(mt, P)], op=ALU.subtract,
                            )

        hT = hTp.tile([P, KC, N], F8, tag="hT")
        qT = qTp.tile([P, KC, N], F16, tag="qT")
        kT = kTp.tile([P, KC, N], F16, tag="kT")
        v_sb = vp.tile([P, MT, HEADS * HD], F8, tag="vv")

        def wpiece(view, n0, pool=None, tag="w"):
            pool = pool or wp
            t = pool.tile([P, KC, 512], F8, tag=tag)
            nc.sync.dma_start(t[:], view[:, :, n0 : n0 + 512])
            return t

        v_pieces = [wpiece(wqkv_v, 2048), wpiece(wqkv_v, 2560)]

        # ---- phase 1: LN1 + transpose -> hT fp8, v GEMM interleaved ----
        # (v(mt) only needs hT[:, :, mt-slice], so it fills the PE while the
        # LN1 chains for later tiles are still on DVE/ACT)
        for mt in range(MT):
            if mt == 0:
                x_t = x_t0
            else:
                x_t = xload.tile([P, D], F32, tag="x_t")
                nc.sync.dma_start(x_t[:, 0:512], x_d.ap()[ts(mt, P), 0:512])
                nc.sync.dma_start(x_t[:, 512:1024], x_d.ap()[ts(mt, P), 512:1024])
            layer_norm_to(x_t[:], hT, None, ln1s, ln1b, mt, ln1_triv)
            ps = bigp.tile([P, N], F32, tag="big")
            for nv in range(2):
                for kk in range(4):
                    nc.tensor.matmul(
                        ps[:, ts(nv, 512)],
                        hT[:, 2 * kk : 2 * kk + 2, ts(mt, P)],
                        v_pieces[nv][:, 2 * kk : 2 * kk + 2, :],
                        start=(kk == 0), stop=(kk == 3),
                        perf_mode=DR, skip_group_check=True,
                    )
            # v bias folds into c1 after proj (normalized probs sum to 1)
            if mt % 2 == 0:
                nc.vector.tensor_scalar(
                    out=v_sb[:, mt, :], in0=ps[:], scalar1=IWS, scalar2=None,
                    op0=ALU.mult,
                )
            else:
                nc.scalar.activation(v_sb[:, mt, :], ps[:], AF.Copy, scale=IWS)

        # ---- phase 2: q/k GEMMs (fp8 DR), nt-outer so the first-half token
        # chains run before the last LN1 tiles land ----
        qk_pieces = [wpiece(wqkv_v, 0), wpiece(wqkv_v, 512),
                     wpiece(wqkv_v, 1024), wpiece(wqkv_v, 1536)]
        for nt in range(2):
            for half in range(2):
                dst_t = qT if half == 0 else kT
                for mc in range(8):
                    piece = qk_pieces[half * 2 + mc // 4]
                    mc_l = mc % 4
                    ps = bigp.tile([P, 512], F32, tag="big")
                    for kk in range(4):
                        nc.tensor.matmul(
                            ps[:],
                            piece[:, 2 * kk : 2 * kk + 2, ts(mc_l, P)],
                            hT[:, 2 * kk : 2 * kk + 2, ts(nt, 512)],
                            start=(kk == 0), stop=(kk == 3),
                            perf_mode=DR, skip_group_check=True,
                        )
                    dst = dst_t[:, mc, ts(nt, 512)]
                    if qk_triv:
                        if mc % 2 == 0:
                            nc.vector.tensor_scalar(
                                out=dst, in0=ps[:], scalar1=IWS, scalar2=None,
                                op0=ALU.mult,
                            )
                        else:
                            nc.scalar.activation(dst, ps[:], AF.Copy, scale=IWS)
                    else:
                        mcg = half * 8 + mc
                        nc.vector.tensor_scalar(
                            out=dst, in0=ps[:], scalar1=IWS,
                            scalar2=bqk[:, mcg : mcg + 1], op0=ALU.mult,
                            op1=ALU.add,
                        )

        # w_proj load early (streams behind attention)
        proj_pieces = [wpiece(wproj_v, 0, wprojp, "wproj"),
                       wpiece(wproj_v, 512, wprojp, "wproj")]

        # ---- phase 3: attention, two query halves ----
        oT = oTp.tile([P, KC, N], F8, tag="oT")

        def scores_exp(h, probs_h, kk, nq):
            """scores psum [128, 2, 512] for mk=2kk,2kk+1 -> exp -> probs fp8."""
            mc = h // 2
            pr = (h % 2) * HD
            sc = bigp.tile([P, 2, 512], F32, tag="big")
            for j in range(2):
                nc.tensor.matmul(
                    sc[:, j, :],
                    kT[pr : pr + HD, mc, ts(2 * kk + j, P)],
                    qT[pr : pr + HD, mc, ts(nq, 512)],
                    start=True, stop=True, skip_group_check=True,
                )
            nc.scalar.activation(
                probs_h[:, 2 * kk : 2 * kk + 2, :], sc[:], AF.Exp,
                bias=ln8_t[:], scale=SCALE,
            )

        for nq in range(2):
            probs_q = []

            def emit_head(h, nq=nq):
                probs_h = probsp.tile([P, MT, 512], F8, tag="probs")
                for kk in range(4):
                    scores_exp(h, probs_h, kk, nq)
                probs_q.append(probs_h)

            emit_head(0)
            emit_head(1)
            for m in range(8):  # head pairs (2m, 2m+1)
                h0, h1 = 2 * m, 2 * m + 1
                p_h0 = probs_q.pop(0)
                p_h1 = probs_q.pop(0)
                pav = pavp.tile([P, 512], F32, tag="pav")
                sps = auxp.tile([16, 512], F32, tag="aux")
                # even head: DR at base 0 (rows 0:64)
                for kk in range(4):
                    nc.tensor.matmul(
                        pav[0:HD, :],
                        v_sb[:, 2 * kk : 2 * kk + 2, h0 * HD : (h0 + 1) * HD],
                        p_h0[:, 2 * kk : 2 * kk + 2, :],
                        start=(kk == 0), stop=(kk == 3),
                        perf_mode=DR, skip_group_check=True,
                    )
                # S for even head (one-hot col 0)
                for kk in range(4):
                    nc.tensor.matmul(
                        sps[:], ohe_t[:],
                        p_h0[:, 2 * kk : 2 * kk + 2, :],
                        start=(kk == 0), stop=False,
                        perf_mode=DR, skip_group_check=True,
                    )
                if m < 7:
                    emit_head(2 * m + 2)
                # odd head: non-DR fp8 at base 64 (rows 64:128)
                for mk in range(MT):
                    nc.tensor.matmul(
                        pav[HD:P, :],
                        v_sb[:, mk, h1 * HD : (h1 + 1) * HD],
                        p_h1[:, mk, :],
                        start=(mk == 0), stop=(mk == MT - 1),
                        skip_group_check=True,
                    )
                # S for odd head (one-hot col 1) closes the pair group
                for kk in range(4):
                    nc.tensor.matmul(
                        sps[:], oho_t[:],
                        p_h1[:, 2 * kk : 2 * kk + 2, :],
                        start=False, stop=(kk == 3),
                        perf_mode=DR, skip_group_check=True,
                    )
                if m < 7:
                    emit_head(2 * m + 3)
                # drain S -> 1/S -> fp16 -> rb broadcast -> oT = pav * rb
                # (rows 2:15 of sps are zero; keep them out of the reciprocal
                # so no inf reaches the rb matmul)
                srow = srowp.tile([16, 512], F32, tag="srow")
                nc.vector.reciprocal(srow[0:2, :], sps[0:2, :])
                srow16 = srowp.tile([16, 512], F16, tag="srow16")
                nc.vector.tensor_copy(srow16[0:2, :], srow[0:2, :])
                rb = auxp.tile([P, 512], F32, tag="aux")
                nc.tensor.matmul(rb[:], sel_t[0:2, :, :], srow16[0:2, :],
                                 start=True, stop=True)
                rb16 = srowp.tile([P, 512], F16, tag="rb16")
                nc.vector.tensor_copy(rb16[:], rb[:])
                nc.vector.tensor_tensor(
                    out=oT[:, m, ts(nq, 512)], in0=pav[:], in1=rb16[:],
                    op=ALU.mult,
                )

        # ---- phase 4+5: proj + residual -> x1, LN2 fused per tile ----
        x1 = x1p.tile([P, MT, D], F16)
        h2hi = h2hip.tile([P, KC, N], F8, tag="h2hi")
        h2lo = h2lop.tile([P, KC, N], F8, tag="h2lo")
        for mt in range(MT):
            x_t = xload.tile([P, D], F32, tag="x_t")
            nc.sync.dma_start(x_t[:], x_d.ap()[ts(mt, P), :])
            ps = bigp.tile([P, N], F32, tag="big")
            for np_ in range(2):
                for kk in range(4):
                    nc.tensor.matmul(
                        ps[:, ts(np_, 512)],
                        oT[:, 2 * kk : 2 * kk + 2, ts(mt, P)],
                        proj_pieces[np_][:, 2 * kk : 2 * kk + 2, :],
                        start=(kk == 0), stop=(kk == 3),
                        perf_mode=DR, skip_group_check=True,
                    )
            nc.vector.scalar_tensor_tensor(
                x1[:, mt, :], ps[:], IWS, x_t[:], ALU.mult, ALU.add,
            )
            if apply_c1:
                nc.vector.tensor_add(x1[:, mt, :], x1[:, mt, :], c1row[:])
            layer_norm_to(x1[:, mt, :], h2hi, h2lo, ln2s, ln2b, mt, ln2_triv)

        # ---- phase 6: fc1 (3-pass fp8 DR) + gelu -> a1 fp8 ----
        # a1 [128, 32, 1024] fp8 reuses qT (chunks 0:16) + kT (16:32) slots
        a1a = qTp.tile([P, 16, N], F8, tag="qT")
        a1b = kTp.tile([P, 16, N], F8, tag="kT")

        for p8 in range(8):  # 512-wide hidden column pieces
            w1h_t = wpiece(w1hi_v, p8 * 512)
            w1l_t = wpiece(w1lo_v, p8 * 512)
            for nt in range(2):
                for mg in range(2):  # two mh chunks per psum
                    ps = bigp.tile([P, 2, 512], F32, tag="big")
                    for ml in range(2):
                        mh_l = mg * 2 + ml
                        for kk in range(4):
                            k2 = slice(2 * kk, 2 * kk + 2)
                            for wt, ht in ((w1h_t, h2hi), (w1l_t, h2hi),
                                           (w1h_t, h2lo)):
                                nc.tensor.matmul(
                                    ps[:, ml, :],
                                    wt[:, k2, ts(mh_l, P)],
                                    ht[:, k2, ts(nt, 512)],
                                    start=(kk == 0 and wt is w1h_t and ht is h2hi),
                                    stop=(kk == 3 and ht is h2lo),
                                    perf_mode=DR, skip_group_check=True,
                                )
                    mhg = p8 * 4 + mg * 2  # first of two mh chunks
                    a1_t = a1a if mhg < 16 else a1b
                    adst = a1_t[:, mhg % 16 : mhg % 16 + 2, ts(nt, 512)]
                    if fc1b_triv:
                        nc.scalar.activation(
                            adst, ps[:], AF.Gelu_apprx_tanh,
                            bias=bfc1[:, 0:1], scale=IWS,
                        )
                    else:
                        for ml in range(2):
                            nc.scalar.activation(
                                a1_t[:, mhg % 16 + ml, ts(nt, 512)],
                                ps[:, ml, :], AF.Gelu_apprx_tanh,
                                bias=bfc1[:, mhg + ml : mhg + ml + 1], scale=IWS,
                            )

        # ---- phase 7: fc2 (1-pass fp8 DR) + residual -> y ----
        # w2 fully resident (32KB fp8, streamed during fc1); one psum
        # accumulation over all 32 hid chunks per token tile -> one stt.
        # (w2 single fp8: measured rel_fro 1.67e-2 vs gate 2e-2.)
        w2h_t = w2pool.tile([P, 32, N], F8, tag="w2h")
        for c8 in range(4):
            nc.sync.dma_start(
                w2h_t[:, c8 * 8 : c8 * 8 + 8, :], w2hi_v[:, c8 * 8 : c8 * 8 + 8, :]
            )
        for mt in range(MT):
            ps = bigp.tile([P, N], F32, tag="big")
            for ncol in range(2):
                for g in range(2):
                    a1_t = a1a if g == 0 else a1b
                    for kk in range(8):
                        a2 = slice(2 * kk, 2 * kk + 2)
                        k2 = slice(g * 16 + 2 * kk, g * 16 + 2 * kk + 2)
                        nc.tensor.matmul(
                            ps[:, ts(ncol, 512)],
                            a1_t[:, a2, ts(mt, P)],
                            w2h_t[:, k2, ts(ncol, 512)],
                            start=(g == 0 and kk == 0), stop=(g == 1 and kk == 7),
                            perf_mode=DR, skip_group_check=True,
                        )
            y_sb = xload.tile([P, D], F16, tag="y_sb")
            nc.vector.scalar_tensor_tensor(
                y_sb[:], ps[:], IWS, x1[:, mt, :], ALU.mult, ALU.add,
            )
            if apply_bfc2:
                nc.vector.tensor_add(y_sb[:], y_sb[:], b2row[:])
            nc.sync.dma_start(y_d.ap()[ts(mt, P), :], y_sb[:])

    nc.compile()
    return nc


_cache = {}


def _get_nc(*key):
    if key not in _cache:
        _cache[key] = build_block(*key)
    return _cache[key]


def _host_consts():
    # rb matmul (non-DR): out[jm, n] = sum_k sel[k, j, m] * srow[k, n] with the
    # lhsT free index (j, m) flattened onto output partitions: partitions 0:64
    # (j=0, even head) read srow row 0; partitions 64:128 (j=1, odd) row 1.
    sel = np.zeros((16, 2, HD), np.float16)
    sel[0, 0, :] = 1.0
    sel[1, 1, :] = 1.0
    ohe = np.zeros((P, 2, 16), np.float32)
    ohe[:, :, 0] = 1.0
    oho = np.zeros((P, 2, 16), np.float32)
    oho[:, :, 1] = 1.0
    return sel, ohe.astype(E4), oho.astype(E4)


def kernel(
    x, w_qkv, b_qkv, w_proj, b_proj, ln1_scale, ln1_bias,
    ln2_scale, ln2_bias, w_fc1, b_fc1, w_fc2, b_fc2,
):
    x = np.asarray(x, np.float32)
    B = x.shape[0]
    b_qkv = np.asarray(b_qkv, np.float32)
    b_v = b_qkv[2 * D :]
    # exact fold: o includes +b_v after softmax-normalize (rows sum to 1),
    # so c1 = b_v @ w_proj + b_proj is a constant row added post-proj.
    c1 = b_v.astype(np.float64) @ np.asarray(w_proj, np.float64) + np.asarray(
        b_proj, np.float64
    )
    c1 = c1.astype(np.float32)
    bfc2 = np.asarray(b_fc2, np.float32)
    b_fc1 = np.asarray(b_fc1, np.float32)
    ln1_scale = np.asarray(ln1_scale, np.float32)
    ln1_bias = np.asarray(ln1_bias, np.float32)
    ln2_scale = np.asarray(ln2_scale, np.float32)
    ln2_bias = np.asarray(ln2_bias, np.float32)
    ln1_triv = bool(np.all(ln1_scale == 1) and np.all(ln1_bias == 0))
    ln2_triv = bool(np.all(ln2_scale == 1) and np.all(ln2_bias == 0))
    qk_triv = bool(np.all(b_qkv[: 2 * D] == 0))
    fc1b_triv = bool(np.all(b_fc1 == 0))
    apply_c1 = bool(np.any(c1 != 0))
    apply_bfc2 = bool(np.any(bfc2 != 0))

    nc = _get_nc(ln1_triv, ln2_triv, qk_triv, fc1b_triv, apply_c1, apply_bfc2)

    def q8(a):
        return np.ascontiguousarray(a).astype(E4)

    w_qkv8 = q8(np.asarray(w_qkv, np.float32) * WS)
    w_proj8 = q8(np.asarray(w_proj, np.float32) * WS)
    w1s = np.asarray(w_fc1, np.float32) * WS
    w1hi = q8(w1s)
    w1lo = q8(w1s - w1hi.astype(np.float32))
    w2s = np.asarray(w_fc2, np.float32) * WS
    w2hi = q8(w2s)
    sel, ohe, oho = _host_consts()

    base = {
        "w_qkv": w_qkv8,
        "w_proj": w_proj8,
        "w1hi": w1hi,
        "w1lo": w1lo,
        "w2hi": w2hi,
        "b_qkv": b_qkv,
        "b_fc1": b_fc1,
        "ln1_scale": ln1_scale,
        "ln1_bias": ln1_bias,
        "ln2_scale": ln2_scale,
        "ln2_bias": ln2_bias,
        "selc": sel,
        "ohe": ohe,
        "oho": oho,
    }
    if apply_c1:
        base["c1"] = c1
    if apply_bfc2:
        base["b_fc2c"] = bfc2

    in_maps = [dict(base, x=np.ascontiguousarray(x[i])) for i in range(B)]
    last_err = None
    for _attempt in range(3):
        try:
            res = run_bass_kernel_spmd(nc, in_maps, core_ids=list(range(B)))
            break
        except Exception as e:  # transient NRT/axon worker failures
            last_err = e
            import time as _time

            _time.sleep(2.0)
    else:
        raise last_err
    out = np.stack([res.results[i]["y"] for i in range(B)], axis=0)
    return np.ascontiguousarray(out.astype(np.float32))


# revision 4
# speedup vs baseline: 1.0283x; 1.0003x over previous
"""Trainium2 Bass kernel for a dense transformer block (pre-LN MHA + MLP).

v2: fp8 e4m3 DoubleRow matmuls for the heavy GEMMs.

Sharding: pure data parallel - batch (8) maps 1:1 onto the 8 NeuronCores.

Quantization scheme (validated vs reference on CPU, rel_fro ~1.2e-2):
  - hT (LN1 out), v, probs (exp out), oT: single fp8, DR matmuls.
  - w_qkv, w_proj: single fp8 (x256 host scale).
  - fc1: 3-pass (h2 hi+lo fp8 x w1 hi+lo fp8; lo*lo term dropped).
  - fc2: w2 hi+lo fp8, a1 single fp8.
  - q/k stored fp16; scores matmuls fp16 (a DR head-dim pack would need
    cross-partition remaps the qkv matmul cannot produce).
  - softmax: probs = exp(s*scale + ln8) unnormalized fp8; S via one-hot
    DR matmuls into a [16, 512] psum per head pair; rb = sel @ fp16(1/S)
    broadcast by the PE; o = pav * rb.
  - DR matmuls must write psum base partition 0 (ISA): odd heads' AV
    runs non-DR fp8 into rows 64:128 of the pair psum.

Self-contained: hardcodes all shapes from the problem spec.
"""

from contextlib import ExitStack

import numpy as np
import ml_dtypes

import concourse.bass as bass
import concourse.tile as tile
from concourse import bacc, mybir
from concourse.bass import ts
from concourse.bass_utils import run_bass_kernel_spmd
from concourse.masks import make_identity

F32 = mybir.dt.float32
F16 = mybir.dt.float16
F8 = mybir.dt.float8e4
AF = mybir.ActivationFunctionType
ALU = mybir.AluOpType
DR = mybir.MatmulPerfMode.DoubleRow
E4 = ml_dtypes.float8_e4m3

P = 128          # partitions
N = 1024         # tokens per core
D = 1024         # model dim
KC = D // P      # 8 contraction chunks of 128
HEADS = 16
HD = 64          # head dim
HID = 4096
EPS = 1e-6
MT = N // P      # 8 token tiles of 128
SCALE = HD ** -0.5
WS = 256.0       # host weight scale
IWS = 1.0 / WS
LN8 = float(np.log(8.0))


def build_block(ln1_triv, ln2_triv, qk_triv, fc1b_triv, apply_c1, apply_bfc2):
    nc = bacc.Bacc("TRN2", target_bir_lowering=False, debug=False, num_devices=8)

    x_d = nc.dram_tensor("x", [N, D], F32, kind="ExternalInput")
    wqkv_d = nc.dram_tensor("w_qkv", [D, 3 * D], F8, kind="ExternalInput")
    wproj_d = nc.dram_tensor("w_proj", [D, D], F8, kind="ExternalInput")
    w1hi_d = nc.dram_tensor("w1hi", [D, HID], F8, kind="ExternalInput")
    w1lo_d = nc.dram_tensor("w1lo", [D, HID], F8, kind="ExternalInput")
    w2hi_d = nc.dram_tensor("w2hi", [HID, D], F8, kind="ExternalInput")
    bqkv_d = nc.dram_tensor("b_qkv", [3 * D], F32, kind="ExternalInput")
    bfc1_d = nc.dram_tensor("b_fc1", [HID], F32, kind="ExternalInput")
    ln1s_d = nc.dram_tensor("ln1_scale", [D], F32, kind="ExternalInput")
    ln1b_d = nc.dram_tensor("ln1_bias", [D], F32, kind="ExternalInput")
    ln2s_d = nc.dram_tensor("ln2_scale", [D], F32, kind="ExternalInput")
    ln2b_d = nc.dram_tensor("ln2_bias", [D], F32, kind="ExternalInput")
    sel_d = nc.dram_tensor("selc", [16, 2, HD], F16, kind="ExternalInput")
    ohe_d = nc.dram_tensor("ohe", [P, 2, 16], F8, kind="ExternalInput")
    oho_d = nc.dram_tensor("oho", [P, 2, 16], F8, kind="ExternalInput")
    c1_d = nc.dram_tensor("c1", [D], F32, kind="ExternalInput") if apply_c1 else None
    bfc2_d = (
        nc.dram_tensor("b_fc2c", [D], F32, kind="ExternalInput") if apply_bfc2 else None
    )
    y_d = nc.dram_tensor("y", [N, D], F16, kind="ExternalOutput")

    # [(kc p), n] -> [p, kc, n] views for weight loads
    wqkv_v = wqkv_d.ap().rearrange("(kc p) n -> p kc n", p=P)
    wproj_v = wproj_d.ap().rearrange("(kc p) n -> p kc n", p=P)
    w1hi_v = w1hi_d.ap().rearrange("(kc p) n -> p kc n", p=P)
    w1lo_v = w1lo_d.ap().rearrange("(kc p) n -> p kc n", p=P)
    w2hi_v = w2hi_d.ap().rearrange("(kc p) n -> p kc n", p=P)

    with tile.TileContext(nc) as tc, ExitStack() as ctx:
        ep = ctx.enter_context
        constp = ep(tc.tile_pool(name="const", bufs=1))
        xload = ep(tc.tile_pool(name="xload", bufs=3))
        htmpp = ep(tc.tile_pool(name="htmp", bufs=3))
        hTp = ep(tc.tile_pool(name="hT", bufs=1))
        h2hip = ep(tc.tile_pool(name="h2hi", bufs=1))
        h2lop = ep(tc.tile_pool(name="h2lo", bufs=1))
        qTp = ep(tc.tile_pool(name="qT", bufs=1))
        kTp = ep(tc.tile_pool(name="kT", bufs=1))
        vp = ep(tc.tile_pool(name="vv", bufs=1))
        oTp = ep(tc.tile_pool(name="oT", bufs=1))
        x1p = ep(tc.tile_pool(name="x1", bufs=1))
        probsp = ep(tc.tile_pool(name="probs", bufs=4))
        wp = ep(tc.tile_pool(name="w", bufs=6))
        wprojp = ep(tc.tile_pool(name="wproj", bufs=2))
        w2pool = ep(tc.tile_pool(name="w2", bufs=1))
        statsp = ep(tc.tile_pool(name="stats", bufs=6))
        srowp = ep(tc.tile_pool(name="srow", bufs=2))
        # PSUM: big [128,1024]x2 (4 banks) + pav [128,512]x2 (2) + aux
        # [128,512]x2 (2) = 8 banks
        bigp = ep(tc.tile_pool(name="big", bufs=2, space="PSUM"))
        pavp = ep(tc.tile_pool(name="pav", bufs=2, space="PSUM"))
        auxp = ep(tc.tile_pool(name="aux", bufs=2, space="PSUM"))

        # ---- first x tile load goes out before anything else ----
        x_t0 = xload.tile([P, D], F32, tag="x_t")
        nc.sync.dma_start(x_t0[:, 0:512], x_d.ap()[ts(0, P), 0:512])
        nc.sync.dma_start(x_t0[:, 512:1024], x_d.ap()[ts(0, P), 512:1024])

        # ---- constants (gpsimd queue; keeps sync queue on x) ----
        ident = constp.tile([P, P], F16)
        make_identity(nc, ident[:])
        eps_t = constp.tile([P, 1], F32)
        nc.vector.memset(eps_t[:], EPS)
        ln8_t = constp.tile([P, 1], F32)
        nc.vector.memset(ln8_t[:], LN8)
        sel_t = constp.tile([16, 2, HD], F16)
        nc.gpsimd.dma_start(sel_t[:], sel_d.ap())
        ohe_t = constp.tile([P, 2, 16], F8)
        nc.gpsimd.dma_start(ohe_t[:], ohe_d.ap())
        oho_t = constp.tile([P, 2, 16], F8)
        nc.gpsimd.dma_start(oho_t[:], oho_d.ap())
        ln1s = constp.tile([P, KC], F32)
        nc.gpsimd.dma_start(ln1s[:], ln1s_d.ap().rearrange("(k p) -> p k", p=P))
        ln1b = constp.tile([P, KC], F32)
        nc.gpsimd.dma_start(ln1b[:], ln1b_d.ap().rearrange("(k p) -> p k", p=P))
        ln2s = constp.tile([P, KC], F32)
        nc.gpsimd.dma_start(ln2s[:], ln2s_d.ap().rearrange("(k p) -> p k", p=P))
        ln2b = constp.tile([P, KC], F32)
        nc.gpsimd.dma_start(ln2b[:], ln2b_d.ap().rearrange("(k p) -> p k", p=P))
        bqk = constp.tile([P, 16], F32)
        bqkv_v = bqkv_d.ap().rearrange("(m p) -> p m", p=P)
        nc.gpsimd.dma_start(bqk[:], bqkv_v[:, 0:16])
        bfc1 = constp.tile([P, HID // P], F32)
        nc.gpsimd.dma_start(bfc1[:], bfc1_d.ap().rearrange("(m p) -> p m", p=P))
        if apply_c1:
            c1row = constp.tile([P, D], F32)
            src = c1_d.ap()
            nc.gpsimd.dma_start(
                c1row[:],
                bass.AP(tensor=src.tensor, offset=src.offset, ap=[[0, P], [1, D]]),
            )
        if apply_bfc2:
            b2row = constp.tile([P, D], F32)
            src = bfc2_d.ap()
            nc.gpsimd.dma_start(
                b2row[:],
                bass.AP(tensor=src.tensor, offset=src.offset, ap=[[0, P], [1, D]]),
            )

        def layer_norm_to(src_ap, out_hi, out_lo, s_cols, b_cols, mt, trivial):
            """LN over free dim of src [128, 1024]; write transposed fp8 into
            out_hi[:, kc, mt*128:...] (+ optional fp8 lo residual)."""
            st = statsp.tile([P, 2, 6], F32, tag="st")
            xr = src_ap.rearrange("p (a b) -> p a b", b=512)
            nc.vector.bn_stats(st[:, 0, :], xr[:, 0, :])
            nc.vector.bn_stats(st[:, 1, :], xr[:, 1, :])
            mv = statsp.tile([P, 2], F32, tag="mv")
            nc.vector.bn_aggr(mv[:], st[:])
            rstd = statsp.tile([P, 1], F32, tag="rstd")
            nc.scalar.activation(rstd[:], mv[:, 1:2], AF.Sqrt, bias=eps_t[:])
            nc.vector.reciprocal(rstd[:], rstd[:])
            h = htmpp.tile([P, D], F16, tag="h")
            nc.vector.tensor_scalar(
                out=h[:, 0:512], in0=src_ap[:, 0:512], scalar1=mv[:, 0:1],
                scalar2=rstd[:], op0=ALU.subtract, op1=ALU.mult,
            )
            nc.gpsimd.tensor_scalar(
                out=h[:, 512:1024], in0=src_ap[:, 512:1024], scalar1=mv[:, 0:1],
                scalar2=rstd[:], op0=ALU.subtract, op1=ALU.mult,
            )
            for kg in range(2):  # transpose batches of 4 kc blocks
                pt_t = auxp.tile([P, 512], F16, tag="aux")
                for kl in range(4):
                    kc = kg * 4 + kl
                    nc.tensor.transpose(pt_t[:, ts(kl, P)], h[:, ts(kc, P)], ident[:])
                dst = out_hi[:, kg * 4 : kg * 4 + 4, ts(mt, P)]
                src4 = pt_t[:].rearrange("p (k c) -> p k c", c=P)
                if trivial:
                    if out_lo is None:
                        # LN1: split copies DVE/ACT
                        if kg == 1:
                            nc.scalar.copy(dst, src4)
                        else:
                            nc.vector.tensor_copy(dst, src4)
                    else:
                        # LN2: ACT writes hi fp8 + full fp16; Pool (sbuf-only)
                        # computes lo so the DVE stays free for proj/stats
                        h216 = htmpp.tile([P, 512], F16, tag="h216")
                        nc.scalar.copy(dst, src4)
                        nc.scalar.copy(h216[:], pt_t[:])
                        nc.gpsimd.tensor_tensor(
                            out=out_lo[:, kg * 4 : kg * 4 + 4, ts(mt, P)],
                            in0=h216[:].rearrange("p (k c) -> p k c", c=P),
                            in1=dst, op=ALU.subtract,
                        )
                else:
                    for kl in range(4):
                        kc = kg * 4 + kl
                        nc.vector.tensor_scalar(
                            out=out_hi[:, kc, ts(mt, P)], in0=pt_t[:, ts(kl, P)],
                            scalar1=s_cols[:, kc : kc + 1],
                            scalar2=b_cols[:, kc : kc + 1],
                            op0=ALU.mult, op1=ALU.add,
                        )
                        if out_lo is not None:
                            tmp = htmpp.tile([P, P], F16, tag="lntmp")
                            nc.vector.tensor_scalar(
                                out=tmp[:], in0=pt_t[:, ts(kl, P)],
                                scalar1=s_cols[:, kc : kc + 1],
                                scalar2=b_cols[:, kc : kc + 1],
                                op0=ALU.mult, op1=ALU.add,
                            )
                            nc.gpsimd.tensor_tensor(
                                out=out_lo[:, kc, ts(mt, P)], in0=tmp[:],
                                in1=out_hi[:, kc, ts(mt, P)], op=ALU.subtract,
                            )

        hT = hTp.tile([P, KC, N], F8, tag="hT")
        qT = qTp.tile([P, KC, N], F16, tag="qT")
        kT = kTp.tile([P, KC, N], F16, tag="kT")
        v_sb = vp.tile([P, MT, HEADS * HD], F8, tag="vv")

        def wpiece(view, n0, pool=None, tag="w"):
            pool = pool or wp
            t = pool.tile([P, KC, 512], F8, tag=tag)
            nc.sync.dma_start(t[:], view[:, :, n0 : n0 + 512])
            return t

        v_pieces = [wpiece(wqkv_v, 2048), wpiece(wqkv_v, 2560)]

        # ---- phase 1: LN1 + transpose -> hT fp8, v GEMM interleaved ----
        # (v(mt) only needs hT[:, :, mt-slice], so it fills the PE while the
        # LN1 chains for later tiles are still on DVE/ACT)
        for mt in range(MT):
            if mt == 0:
                x_t = x_t0
            else:
                x_t = xload.tile([P, D], F32, tag="x_t")
                nc.sync.dma_start(x_t[:, 0:512], x_d.ap()[ts(mt, P), 0:512])
                nc.sync.dma_start(x_t[:, 512:1024], x_d.ap()[ts(mt, P), 512:1024])
            layer_norm_to(x_t[:], hT, None, ln1s, ln1b, mt, ln1_triv)
            ps = bigp.tile([P, N], F32, tag="big")
            for nv in range(2):
                for kk in range(4):
                    nc.tensor.matmul(
                        ps[:, ts(nv, 512)],
                        hT[:, 2 * kk : 2 * kk + 2, ts(mt, P)],
                        v_pieces[nv][:, 2 * kk : 2 * kk + 2, :],
                        start=(kk == 0), stop=(kk == 3),
                        perf_mode=DR, skip_group_check=True,
                    )
            # v bias folds into c1 after proj (normalized probs sum to 1)
            if mt % 2 == 0:
                nc.vector.tensor_scalar(
                    out=v_sb[:, mt, :], in0=ps[:], scalar1=IWS, scalar2=None,
                    op0=ALU.mult,
                )
            else:
                nc.scalar.activation(v_sb[:, mt, :], ps[:], AF.Copy, scale=IWS)

        # ---- phase 2: q/k GEMMs (fp8 DR), nt-outer so the first-half token
        # chains run before the last LN1 tiles land ----
        qk_pieces = [wpiece(wqkv_v, 0), wpiece(wqkv_v, 512),
                     wpiece(wqkv_v, 1024), wpiece(wqkv_v, 1536)]
        for nt in range(2):
            for half in range(2):
                dst_t = qT if half == 0 else kT
                for mc in range(8):
                    piece = qk_pieces[half * 2 + mc // 4]
                    mc_l = mc % 4
                    ps = bigp.tile([P, 512], F32, tag="big")
                    for kk in range(4):
                        nc.tensor.matmul(
                            ps[:],
                            piece[:, 2 * kk : 2 * kk + 2, ts(mc_l, P)],
                            hT[:, 2 * kk : 2 * kk + 2, ts(nt, 512)],
                            start=(kk == 0), stop=(kk == 3),
                            perf_mode=DR, skip_group_check=True,
                        )
                    dst = dst_t[:, mc, ts(nt, 512)]
                    if qk_triv:
                        if mc % 2 == 0:
                            nc.vector.tensor_scalar(
                                out=dst, in0=ps[:], scalar1=IWS, scalar2=None,
                                op0=ALU.mult,
                            )
                        else:
                            nc.scalar.activation(dst, ps[:], AF.Copy, scale=IWS)
                    else:
                        mcg = half * 8 + mc
                        nc.vector.tensor_scalar(
                            out=dst, in0=ps[:], scalar1=IWS,
                            scalar2=bqk[:, mcg : mcg + 1], op0=ALU.mult,
                            op1=ALU.add,
                        )

        # w_proj load early (streams behind attention)
        proj_pieces = [wpiece(wproj_v, 0, wprojp, "wproj"),
                       wpiece(wproj_v, 512, wprojp, "wproj")]

        # ---- phase 3: attention, two query halves ----
        oT = oTp.tile([P, KC, N], F8, tag="oT")

        def scores_exp(h, probs_h, kk, nq):
            """scores psum [128, 2, 512] for mk=2kk,2kk+1 -> exp -> probs fp8."""
            mc = h // 2
            pr = (h % 2) * HD
            sc = bigp.tile([P, 2, 512], F32, tag="big")
            for j in range(2):
                nc.tensor.matmul(
                    sc[:, j, :],
                    kT[pr : pr + HD, mc, ts(2 * kk + j, P)],
                    qT[pr : pr + HD, mc, ts(nq, 512)],
                    start=True, stop=True, skip_group_check=True,
                )
            nc.scalar.activation(
                probs_h[:, 2 * kk : 2 * kk + 2, :], sc[:], AF.Exp,
                bias=ln8_t[:], scale=SCALE,
            )

        for nq in range(2):
            probs_q = []

            def emit_head(h, nq=nq):
                probs_h = probsp.tile([P, MT, 512], F8, tag="probs")
                for kk in range(4):
                    scores_exp(h, probs_h, kk, nq)
                probs_q.append(probs_h)

            emit_head(0)
            emit_head(1)
            for m in range(8):  # head pairs (2m, 2m+1)
                h0, h1 = 2 * m, 2 * m + 1
                p_h0 = probs_q.pop(0)
                p_h1 = probs_q.pop(0)
                pav = pavp.tile([P, 512], F32, tag="pav")
                sps = auxp.tile([16, 512], F32, tag="aux")
                # even head: DR at base 0 (rows 0:64)
                for kk in range(4):
                    nc.tensor.matmul(
                        pav[0:HD, :],
                        v_sb[:, 2 * kk : 2 * kk + 2, h0 * HD : (h0 + 1) * HD],
                        p_h0[:, 2 * kk : 2 * kk + 2, :],
                        start=(kk == 0), stop=(kk == 3),
                        perf_mode=DR, skip_group_check=True,
                    )
                # S for even head (one-hot col 0)
                for kk in range(4):
                    nc.tensor.matmul(
                        sps[:], ohe_t[:],
                        p_h0[:, 2 * kk : 2 * kk + 2, :],
                        start=(kk == 0), stop=False,
                        perf_mode=DR, skip_group_check=True,
                    )
                if m < 7:
                    emit_head(2 * m + 2)
                # odd head: non-DR fp8 at base 64 (rows 64:128)
                for mk in range(MT):
                    nc.tensor.matmul(
                        pav[HD:P, :],
                        v_sb[:, mk, h1 * HD : (h1 + 1) * HD],
                        p_h1[:, mk, :],
                        start=(mk == 0), stop=(mk == MT - 1),
                        skip_group_check=True,
                    )
                # S for odd head (one-hot col 1) closes the pair group
                for kk in range(4):
                    nc.tensor.matmul(
                        sps[:], oho_t[:],
                        p_h1[:, 2 * kk : 2 * kk + 2, :],
                        start=False, stop=(kk == 3),
                        perf_mode=DR, skip_group_check=True,
                    )
                if m < 7:
                    emit_head(2 * m + 3)
                # drain S -> 1/S -> fp16 -> rb broadcast -> oT = pav * rb
                # (rows 2:15 of sps are zero; keep them out of the reciprocal
                # so no inf reaches the rb matmul)
                srow = srowp.tile([16, 512], F32, tag="srow")
                nc.vector.reciprocal(srow[0:2, :], sps[0:2, :])
                srow16 = srowp.tile([16, 512], F16, tag="srow16")
                nc.vector.tensor_copy(srow16[0:2, :], srow[0:2, :])
                rb = auxp.tile([P, 512], F32, tag="aux")
                nc.tensor.matmul(rb[:], sel_t[0:2, :, :], srow16[0:2, :],
                                 start=True, stop=True)
                rb16 = srowp.tile([P, 512], F16, tag="rb16")
                nc.vector.tensor_copy(rb16[:], rb[:])
                nc.vector.tensor_tensor(
                    out=oT[:, m, ts(nq, 512)], in0=pav[:], in1=rb16[:],
                    op=ALU.mult,
                )

        # ---- phase 4+5: proj + residual -> x1, LN2 fused per tile ----
        x1 = x1p.tile([P, MT, D], F16)
        h2hi = h2hip.tile([P, KC, N], F8, tag="h2hi")
        h2lo = h2lop.tile([P, KC, N], F8, tag="h2lo")
        for mt in range(MT):
            x_t = xload.tile([P, D], F32, tag="x_t")
            nc.sync.dma_start(x_t[:], x_d.ap()[ts(mt, P), :])
            ps = bigp.tile([P, N], F32, tag="big")
            for np_ in range(2):
                for kk in range(4):
                    nc.tensor.matmul(
                        ps[:, ts(np_, 512)],
                        oT[:, 2 * kk : 2 * kk + 2, ts(mt, P)],
                        proj_pieces[np_][:, 2 * kk : 2 * kk + 2, :],
                        start=(kk == 0), stop=(kk == 3),
                        perf_mode=DR, skip_group_check=True,
                    )
            nc.vector.scalar_tensor_tensor(
                x1[:, mt, :], ps[:], IWS, x_t[:], ALU.mult, ALU.add,
            )
            if apply_c1:
                nc.vector.tensor_add(x1[:, mt, :], x1[:, mt, :], c1row[:])
            layer_norm_to(x1[:, mt, :], h2hi, h2lo, ln2s, ln2b, mt, ln2_triv)

        # ---- phase 6: fc1 (3-pass fp8 DR) + gelu -> a1 fp8 ----
        # a1 [128, 32, 1024] fp8 reuses qT (chunks 0:16) + kT (16:32) slots
        a1a = qTp.tile([P, 16, N], F8, tag="qT")
        a1b = kTp.tile([P, 16, N], F8, tag="kT")

        for p8 in range(8):  # 512-wide hidden column pieces
            w1h_t = wpiece(w1hi_v, p8 * 512)
            w1l_t = wpiece(w1lo_v, p8 * 512)
            for nt in range(2):
                for mg in range(2):  # two mh chunks per psum
                    ps = bigp.tile([P, 2, 512], F32, tag="big")
                    for ml in range(2):
                        mh_l = mg * 2 + ml
                        for kk in range(4):
                            k2 = slice(2 * kk, 2 * kk + 2)
                            for wt, ht in ((w1h_t, h2hi), (w1l_t, h2hi),
                                           (w1h_t, h2lo)):
                                nc.tensor.matmul(
                                    ps[:, ml, :],
                                    wt[:, k2, ts(mh_l, P)],
                                    ht[:, k2, ts(nt, 512)],
                                    start=(kk == 0 and wt is w1h_t and ht is h2hi),
                                    stop=(kk == 3 and ht is h2lo),
                                    perf_mode=DR, skip_group_check=True,
                                )
                    mhg = p8 * 4 + mg * 2  # first of two mh chunks
                    a1_t = a1a if mhg < 16 else a1b
                    adst = a1_t[:, mhg % 16 : mhg % 16 + 2, ts(nt, 512)]
                    if fc1b_triv:
                        nc.scalar.activation(
                            adst, ps[:], AF.Gelu_apprx_tanh,
                            bias=bfc1[:, 0:1], scale=IWS,
                        )
                    else:
                        for ml in range(2):
                            nc.scalar.activation(
                                a1_t[:, mhg % 16 + ml, ts(nt, 512)],
                                ps[:, ml, :], AF.Gelu_apprx_tanh,
                                bias=bfc1[:, mhg + ml : mhg + ml + 1], scale=IWS,
                            )

        # ---- phase 7: fc2 (1-pass fp8 DR) + residual -> y ----
        # w2 fully resident (32KB fp8, streamed during fc1); one psum
        # accumulation over all 32 hid chunks per token tile -> one stt.
        # (w2 single fp8: measured rel_fro 1.67e-2 vs gate 2e-2.)
        w2h_t = w2pool.tile([P, 32, N], F8, tag="w2h")
        for c8 in range(4):
            nc.sync.dma_start(
                w2h_t[:, c8 * 8 : c8 * 8 + 8, :], w2hi_v[:, c8 * 8 : c8 * 8 + 8, :]
            )
        for mt in range(MT):
            ps = bigp.tile([P, N], F32, tag="big")
            for ncol in range(2):
                for g in range(2):
                    a1_t = a1a if g == 0 else a1b
                    for kk in range(8):
                        a2 = slice(2 * kk, 2 * kk + 2)
                        k2 = slice(g * 16 + 2 * kk, g * 16 + 2 * kk + 2)
                        nc.tensor.matmul(
                            ps[:, ts(ncol, 512)],
                            a1_t[:, a2, ts(mt, P)],
                            w2h_t[:, k2, ts(ncol, 512)],
                            start=(g == 0 and kk == 0), stop=(g == 1 and kk == 7),
                            perf_mode=DR, skip_group_check=True,
                        )
            y_sb = xload.tile([P, D], F16, tag="y_sb")
            nc.vector.scalar_tensor_tensor(
                y_sb[:], ps[:], IWS, x1[:, mt, :], ALU.mult, ALU.add,
            )
            if apply_bfc2:
                nc.vector.tensor_add(y_sb[:], y_sb[:], b2row[:])
            nc.sync.dma_start(y_d.ap()[ts(mt, P), :], y_sb[:])

    nc.compile()
    return nc


_cache = {}


def _get_nc(*key):
    if key not in _cache:
        _cache[key] = build_block(*key)
    return _cache[key]


def _host_consts():
    # rb matmul (non-DR): out[jm, n] = sum_k sel[k, j, m] * srow[k, n] with the
    # lhsT free index (j, m) flattened onto output partitions: partitions 0:64
    # (j=0, even head) read srow row 0; partitions 64:128 (j=1, odd) row 1.
    sel = np.zeros((16, 2, HD), np.float16)
    sel[0, 0, :] = 1.0
    sel[1, 1, :] = 1.0
    ohe = np.zeros((P, 2, 16), np.float32)
    ohe[:, :, 0] = 1.0
    oho = np.zeros((P, 2, 16), np.float32)
    oho[:, :, 1] = 1.0
    return sel, ohe.astype(E4), oho.astype(E4)


def kernel(
    x, w_qkv, b_qkv, w_proj, b_proj, ln1_scale, ln1_bias,
    ln2_scale, ln2_bias, w_fc1, b_fc1, w_fc2, b_fc2,
):
    x = np.asarray(x, np.float32)
    B = x.shape[0]
    b_qkv = np.asarray(b_qkv, np.float32)
    b_v = b_qkv[2 * D :]
    # exact fold: o includes +b_v after softmax-normalize (rows sum to 1),
    # so c1 = b_v @ w_proj + b_proj is a constant row added post-proj.
    c1 = b_v.astype(np.float64) @ np.asarray(w_proj, np.float64) + np.asarray(
        b_proj, np.float64
    )
    c1 = c1.astype(np.float32)
    bfc2 = np.asarray(b_fc2, np.float32)
    b_fc1 = np.asarray(b_fc1, np.float32)
    ln1_scale = np.asarray(ln1_scale, np.float32)
    ln1_bias = np.asarray(ln1_bias, np.float32)
    ln2_scale = np.asarray(ln2_scale, np.float32)
    ln2_bias = np.asarray(ln2_bias, np.float32)
    ln1_triv = bool(np.all(ln1_scale == 1) and np.all(ln1_bias == 0))
    ln2_triv = bool(np.all(ln2_scale == 1) and np.all(ln2_bias == 0))
    qk_triv = bool(np.all(b_qkv[: 2 * D] == 0))
    fc1b_triv = bool(np.all(b_fc1 == 0))
    apply_c1 = bool(np.any(c1 != 0))
    apply_bfc2 = bool(np.any(bfc2 != 0))

    nc = _get_nc(ln1_triv, ln2_triv, qk_triv, fc1b_triv, apply_c1, apply_bfc2)

    def q8(a):
        return np.ascontiguousarray(a).astype(E4)

    w_qkv8 = q8(np.asarray(w_qkv, np.float32) * WS)
    w_proj8 = q8(np.asarray(w_proj, np.float32) * WS)
    w1s = np.asarray(w_fc1, np.float32) * WS
    w1hi = q8(w1s)
    w1lo = q8(w1s - w1hi.astype(np.float32))
    w2s = np.asarray(w_fc2, np.float32) * WS
    w2hi = q8(w2s)
    sel, ohe, oho = _host_consts()

    base = {
        "w_qkv": w_qkv8,
        "w_proj": w_proj8,
        "w1hi": w1hi,
        "w1lo": w1lo,
        "w2hi": w2hi,
        "b_qkv": b_qkv,
        "b_fc1": b_fc1,
        "ln1_scale": ln1_scale,
        "ln1_bias": ln1_bias,
        "ln2_scale": ln2_scale,
        "ln2_bias": ln2_bias,
        "selc": sel,
        "ohe": ohe,
        "oho": oho,
    }
    if apply_c1:
        base["c1"] = c1
    if apply_bfc2:
        base["b_fc2c"] = bfc2

    in_maps = [dict(base, x=np.ascontiguousarray(x[i])) for i in range(B)]
    last_err = None
    for _attempt in range(3):
        try:
            res = run_bass_kernel_spmd(nc, in_maps, core_ids=list(range(B)))
            break
        except Exception as e:  # transient NRT/axon worker failures
            last_err = e
            import time as _time

            _time.sleep(2.0)
    else:
        raise last_err
    out = np.stack([res.results[i]["y"] for i in range(B)], axis=0)
    return np.ascontiguousarray(out.astype(np.float32))


# revision 5
# speedup vs baseline: 1.0287x; 1.0004x over previous
"""Trainium2 Bass kernel for a dense transformer block (pre-LN MHA + MLP).

v2: fp8 e4m3 DoubleRow matmuls for the heavy GEMMs.

Sharding: pure data parallel - batch (8) maps 1:1 onto the 8 NeuronCores.

Quantization scheme (validated vs reference on CPU, rel_fro ~1.2e-2):
  - hT (LN1 out), v, probs (exp out), oT: single fp8, DR matmuls.
  - w_qkv, w_proj: single fp8 (x256 host scale).
  - fc1: 3-pass (h2 hi+lo fp8 x w1 hi+lo fp8; lo*lo term dropped).
  - fc2: w2 hi+lo fp8, a1 single fp8.
  - q/k stored fp16; scores matmuls fp16 (a DR head-dim pack would need
    cross-partition remaps the qkv matmul cannot produce).
  - softmax: probs = exp(s*scale + ln8) unnormalized fp8; S via one-hot
    DR matmuls into a [16, 512] psum per head pair; rb = sel @ fp16(1/S)
    broadcast by the PE; o = pav * rb.
  - DR matmuls must write psum base partition 0 (ISA): odd heads' AV
    runs non-DR fp8 into rows 64:128 of the pair psum.

Self-contained: hardcodes all shapes from the problem spec.
"""

from contextlib import ExitStack

import numpy as np
import ml_dtypes

import concourse.bass as bass
import concourse.tile as tile
from concourse import bacc, mybir
from concourse.bass import ts
from concourse.bass_utils import run_bass_kernel_spmd
from concourse.masks import make_identity

F32 = mybir.dt.float32
F16 = mybir.dt.float16
F8 = mybir.dt.float8e4
AF = mybir.ActivationFunctionType
ALU = mybir.AluOpType
DR = mybir.MatmulPerfMode.DoubleRow
E4 = ml_dtypes.float8_e4m3

P = 128          # partitions
N = 1024         # tokens per core
D = 1024         # model dim
KC = D // P      # 8 contraction chunks of 128
HEADS = 16
HD = 64          # head dim
HID = 4096
EPS = 1e-6
MT = N // P      # 8 token tiles of 128
SCALE = HD ** -0.5
WS = 256.0       # host weight scale
IWS = 1.0 / WS
LN8 = float(np.log(8.0))


def build_block(ln1_triv, ln2_triv, qk_triv, fc1b_triv, apply_c1, apply_bfc2):
    nc = bacc.Bacc("TRN2", target_bir_lowering=False, debug=False, num_devices=8)

    x_d = nc.dram_tensor("x", [N, D], F32, kind="ExternalInput")
    wqkv_d = nc.dram_tensor("w_qkv", [D, 3 * D], F8, kind="ExternalInput")
    wproj_d = nc.dram_tensor("w_proj", [D, D], F8, kind="ExternalInput")
    w1hi_d = nc.dram_tensor("w1hi", [D, HID], F8, kind="ExternalInput")
    w1lo_d = nc.dram_tensor("w1lo", [D, HID], F8, kind="ExternalInput")
    w2hi_d = nc.dram_tensor("w2hi", [HID, D], F8, kind="ExternalInput")
    bqkv_d = nc.dram_tensor("b_qkv", [3 * D], F32, kind="ExternalInput")
    bfc1_d = nc.dram_tensor("b_fc1", [HID], F32, kind="ExternalInput")
    ln1s_d = nc.dram_tensor("ln1_scale", [D], F32, kind="ExternalInput")
    ln1b_d = nc.dram_tensor("ln1_bias", [D], F32, kind="ExternalInput")
    ln2s_d = nc.dram_tensor("ln2_scale", [D], F32, kind="ExternalInput")
    ln2b_d = nc.dram_tensor("ln2_bias", [D], F32, kind="ExternalInput")
    sel_d = nc.dram_tensor("selc", [16, 2, HD], F16, kind="ExternalInput")
    ohe_d = nc.dram_tensor("ohe", [P, 2, 16], F8, kind="ExternalInput")
    oho_d = nc.dram_tensor("oho", [P, 2, 16], F8, kind="ExternalInput")
    c1_d = nc.dram_tensor("c1", [D], F32, kind="ExternalInput") if apply_c1 else None
    bfc2_d = (
        nc.dram_tensor("b_fc2c", [D], F32, kind="ExternalInput") if apply_bfc2 else None
    )
    y_d = nc.dram_tensor("y", [N, D], F16, kind="ExternalOutput")

    # [(kc p), n] -> [p, kc, n] views for weight loads
    wqkv_v = wqkv_d.ap().rearrange("(kc p) n -> p kc n", p=P)
    wproj_v = wproj_d.ap().rearrange("(kc p) n -> p kc n", p=P)
    w1hi_v = w1hi_d.ap().rearrange("(kc p) n -> p kc n", p=P)
    w1lo_v = w1lo_d.ap().rearrange("(kc p) n -> p kc n", p=P)
    w2hi_v = w2hi_d.ap().rearrange("(kc p) n -> p kc n", p=P)

    with tile.TileContext(nc) as tc, ExitStack() as ctx:
        ep = ctx.enter_context
        constp = ep(tc.tile_pool(name="const", bufs=1))
        xload = ep(tc.tile_pool(name="xload", bufs=3))
        htmpp = ep(tc.tile_pool(name="htmp", bufs=4))
        hTp = ep(tc.tile_pool(name="hT", bufs=1))
        h2hip = ep(tc.tile_pool(name="h2hi", bufs=1))
        h2lop = ep(tc.tile_pool(name="h2lo", bufs=1))
        qTp = ep(tc.tile_pool(name="qT", bufs=1))
        kTp = ep(tc.tile_pool(name="kT", bufs=1))
        vp = ep(tc.tile_pool(name="vv", bufs=1))
        oTp = ep(tc.tile_pool(name="oT", bufs=1))
        x1p = ep(tc.tile_pool(name="x1", bufs=1))
        probsp = ep(tc.tile_pool(name="probs", bufs=4))
        wp = ep(tc.tile_pool(name="w", bufs=6))
        wprojp = ep(tc.tile_pool(name="wproj", bufs=2))
        w2pool = ep(tc.tile_pool(name="w2", bufs=1))
        statsp = ep(tc.tile_pool(name="stats", bufs=6))
        srowp = ep(tc.tile_pool(name="srow", bufs=2))
        # PSUM: big [128,1024]x2 (4 banks) + pav [128,512]x2 (2) + aux
        # [128,512]x2 (2) = 8 banks
        bigp = ep(tc.tile_pool(name="big", bufs=2, space="PSUM"))
        pavp = ep(tc.tile_pool(name="pav", bufs=2, space="PSUM"))
        auxp = ep(tc.tile_pool(name="aux", bufs=2, space="PSUM"))

        # ---- first x tile load goes out before anything else ----
        x_t0 = xload.tile([P, D], F32, tag="x_t")
        nc.sync.dma_start(x_t0[:, 0:512], x_d.ap()[ts(0, P), 0:512])
        nc.sync.dma_start(x_t0[:, 512:1024], x_d.ap()[ts(0, P), 512:1024])

        # ---- constants (gpsimd queue; keeps sync queue on x) ----
        ident = constp.tile([P, P], F16)
        make_identity(nc, ident[:])
        eps_t = constp.tile([P, 1], F32)
        nc.vector.memset(eps_t[:], EPS)
        ln8_t = constp.tile([P, 1], F32)
        nc.vector.memset(ln8_t[:], LN8)
        sel_t = constp.tile([16, 2, HD], F16)
        nc.gpsimd.dma_start(sel_t[:], sel_d.ap())
        ohe_t = constp.tile([P, 2, 16], F8)
        nc.gpsimd.dma_start(ohe_t[:], ohe_d.ap())
        oho_t = constp.tile([P, 2, 16], F8)
        nc.gpsimd.dma_start(oho_t[:], oho_d.ap())
        ln1s = constp.tile([P, KC], F32)
        nc.gpsimd.dma_start(ln1s[:], ln1s_d.ap().rearrange("(k p) -> p k", p=P))
        ln1b = constp.tile([P, KC], F32)
        nc.gpsimd.dma_start(ln1b[:], ln1b_d.ap().rearrange("(k p) -> p k", p=P))
        ln2s = constp.tile([P, KC], F32)
        nc.gpsimd.dma_start(ln2s[:], ln2s_d.ap().rearrange("(k p) -> p k", p=P))
        ln2b = constp.tile([P, KC], F32)
        nc.gpsimd.dma_start(ln2b[:], ln2b_d.ap().rearrange("(k p) -> p k", p=P))
        bqk = constp.tile([P, 16], F32)
        bqkv_v = bqkv_d.ap().rearrange("(m p) -> p m", p=P)
        nc.gpsimd.dma_start(bqk[:], bqkv_v[:, 0:16])
        bfc1 = constp.tile([P, HID // P], F32)
        nc.gpsimd.dma_start(bfc1[:], bfc1_d.ap().rearrange("(m p) -> p m", p=P))
        if apply_c1:
            c1row = constp.tile([P, D], F32)
            src = c1_d.ap()
            nc.gpsimd.dma_start(
                c1row[:],
                bass.AP(tensor=src.tensor, offset=src.offset, ap=[[0, P], [1, D]]),
            )
        if apply_bfc2:
            b2row = constp.tile([P, D], F32)
            src = bfc2_d.ap()
            nc.gpsimd.dma_start(
                b2row[:],
                bass.AP(tensor=src.tensor, offset=src.offset, ap=[[0, P], [1, D]]),
            )

        def layer_norm_to(src_ap, out_hi, out_lo, s_cols, b_cols, mt, trivial):
            """LN over free dim of src [128, 1024]; write transposed fp8 into
            out_hi[:, kc, mt*128:...] (+ optional fp8 lo residual)."""
            st = statsp.tile([P, 2, 6], F32, tag="st")
            xr = src_ap.rearrange("p (a b) -> p a b", b=512)
            nc.vector.bn_stats(st[:, 0, :], xr[:, 0, :])
            nc.vector.bn_stats(st[:, 1, :], xr[:, 1, :])
            mv = statsp.tile([P, 2], F32, tag="mv")
            nc.vector.bn_aggr(mv[:], st[:])
            rstd = statsp.tile([P, 1], F32, tag="rstd")
            nc.scalar.activation(rstd[:], mv[:, 1:2], AF.Sqrt, bias=eps_t[:])
            nc.vector.reciprocal(rstd[:], rstd[:])
            h = htmpp.tile([P, D], F16, tag="h")
            nc.vector.tensor_scalar(
                out=h[:, 0:512], in0=src_ap[:, 0:512], scalar1=mv[:, 0:1],
                scalar2=rstd[:], op0=ALU.subtract, op1=ALU.mult,
            )
            nc.gpsimd.tensor_scalar(
                out=h[:, 512:1024], in0=src_ap[:, 512:1024], scalar1=mv[:, 0:1],
                scalar2=rstd[:], op0=ALU.subtract, op1=ALU.mult,
            )
            for kg in range(2):  # transpose batches of 4 kc blocks
                pt_t = auxp.tile([P, 512], F16, tag="aux")
                for kl in range(4):
                    kc = kg * 4 + kl
                    nc.tensor.transpose(pt_t[:, ts(kl, P)], h[:, ts(kc, P)], ident[:])
                dst = out_hi[:, kg * 4 : kg * 4 + 4, ts(mt, P)]
                src4 = pt_t[:].rearrange("p (k c) -> p k c", c=P)
                if trivial:
                    if out_lo is None:
                        # LN1: split copies DVE/ACT
                        if kg == 1:
                            nc.scalar.copy(dst, src4)
                        else:
                            nc.vector.tensor_copy(dst, src4)
                    else:
                        # LN2: ACT writes hi fp8 + full fp16; Pool (sbuf-only)
                        # computes lo so the DVE stays free for proj/stats
                        h216 = htmpp.tile([P, 512], F16, tag="h216")
                        nc.scalar.copy(dst, src4)
                        nc.scalar.copy(h216[:], pt_t[:])
                        nc.gpsimd.tensor_tensor(
                            out=out_lo[:, kg * 4 : kg * 4 + 4, ts(mt, P)],
                            in0=h216[:].rearrange("p (k c) -> p k c", c=P),
                            in1=dst, op=ALU.subtract,
                        )
                else:
                    for kl in range(4):
                        kc = kg * 4 + kl
                        nc.vector.tensor_scalar(
                            out=out_hi[:, kc, ts(mt, P)], in0=pt_t[:, ts(kl, P)],
                            scalar1=s_cols[:, kc : kc + 1],
                            scalar2=b_cols[:, kc : kc + 1],
                            op0=ALU.mult, op1=ALU.add,
                        )
                        if out_lo is not None:
                            tmp = htmpp.tile([P, P], F16, tag="lntmp")
                            nc.vector.tensor_scalar(
                                out=tmp[:], in0=pt_t[:, ts(kl, P)],
                                scalar1=s_cols[:, kc : kc + 1],
                                scalar2=b_cols[:, kc : kc + 1],
                                op0=ALU.mult, op1=ALU.add,
                            )
                            nc.gpsimd.tensor_tensor(
                                out=out_lo[:, kc, ts(mt, P)], in0=tmp[:],
                                in1=out_hi[:, kc, ts(mt, P)], op=ALU.subtract,
                            )

        hT = hTp.tile([P, KC, N], F8, tag="hT")
        qT = qTp.tile([P, KC, N], F16, tag="qT")
        kT = kTp.tile([P, KC, N], F16, tag="kT")
        v_sb = vp.tile([P, MT, HEADS * HD], F8, tag="vv")

        def wpiece(view, n0, pool=None, tag="w"):
            pool = pool or wp
            t = pool.tile([P, KC, 512], F8, tag=tag)
            nc.sync.dma_start(t[:], view[:, :, n0 : n0 + 512])
            return t

        v_pieces = [wpiece(wqkv_v, 2048), wpiece(wqkv_v, 2560)]

        # ---- phase 1: LN1 + transpose -> hT fp8, v GEMM interleaved ----
        # (v(mt) only needs hT[:, :, mt-slice], so it fills the PE while the
        # LN1 chains for later tiles are still on DVE/ACT)
        for mt in range(MT):
            if mt == 0:
                x_t = x_t0
            else:
                x_t = xload.tile([P, D], F32, tag="x_t")
                nc.sync.dma_start(x_t[:, 0:512], x_d.ap()[ts(mt, P), 0:512])
                nc.sync.dma_start(x_t[:, 512:1024], x_d.ap()[ts(mt, P), 512:1024])
            layer_norm_to(x_t[:], hT, None, ln1s, ln1b, mt, ln1_triv)
            ps = bigp.tile([P, N], F32, tag="big")
            for nv in range(2):
                for kk in range(4):
                    nc.tensor.matmul(
                        ps[:, ts(nv, 512)],
                        hT[:, 2 * kk : 2 * kk + 2, ts(mt, P)],
                        v_pieces[nv][:, 2 * kk : 2 * kk + 2, :],
                        start=(kk == 0), stop=(kk == 3),
                        perf_mode=DR, skip_group_check=True,
                    )
            # v bias folds into c1 after proj (normalized probs sum to 1)
            if mt % 2 == 0:
                nc.vector.tensor_scalar(
                    out=v_sb[:, mt, :], in0=ps[:], scalar1=IWS, scalar2=None,
                    op0=ALU.mult,
                )
            else:
                nc.scalar.activation(v_sb[:, mt, :], ps[:], AF.Copy, scale=IWS)

        # ---- phase 2: q/k GEMMs (fp8 DR), nt-outer so the first-half token
        # chains run before the last LN1 tiles land ----
        qk_pieces = [wpiece(wqkv_v, 0), wpiece(wqkv_v, 512),
                     wpiece(wqkv_v, 1024), wpiece(wqkv_v, 1536)]
        for nt in range(2):
            for half in range(2):
                dst_t = qT if half == 0 else kT
                for mc in range(8):
                    piece = qk_pieces[half * 2 + mc // 4]
                    mc_l = mc % 4
                    ps = bigp.tile([P, 512], F32, tag="big")
                    for kk in range(4):
                        nc.tensor.matmul(
                            ps[:],
                            piece[:, 2 * kk : 2 * kk + 2, ts(mc_l, P)],
                            hT[:, 2 * kk : 2 * kk + 2, ts(nt, 512)],
                            start=(kk == 0), stop=(kk == 3),
                            perf_mode=DR, skip_group_check=True,
                        )
                    dst = dst_t[:, mc, ts(nt, 512)]
                    if qk_triv:
                        if mc % 2 == 0:
                            nc.vector.tensor_scalar(
                                out=dst, in0=ps[:], scalar1=IWS, scalar2=None,
                                op0=ALU.mult,
                            )
                        else:
                            nc.scalar.activation(dst, ps[:], AF.Copy, scale=IWS)
                    else:
                        mcg = half * 8 + mc
                        nc.vector.tensor_scalar(
                            out=dst, in0=ps[:], scalar1=IWS,
                            scalar2=bqk[:, mcg : mcg + 1], op0=ALU.mult,
                            op1=ALU.add,
                        )

        # w_proj load early (streams behind attention)
        proj_pieces = [wpiece(wproj_v, 0, wprojp, "wproj"),
                       wpiece(wproj_v, 512, wprojp, "wproj")]

        # ---- phase 3: attention, two query halves ----
        oT = oTp.tile([P, KC, N], F8, tag="oT")

        def scores_exp(h, probs_h, kk, nq):
            """scores psum [128, 2, 512] for mk=2kk,2kk+1 -> exp -> probs fp8."""
            mc = h // 2
            pr = (h % 2) * HD
            sc = bigp.tile([P, 2, 512], F32, tag="big")
            for j in range(2):
                nc.tensor.matmul(
                    sc[:, j, :],
                    kT[pr : pr + HD, mc, ts(2 * kk + j, P)],
                    qT[pr : pr + HD, mc, ts(nq, 512)],
                    start=True, stop=True, skip_group_check=True,
                )
            nc.scalar.activation(
                probs_h[:, 2 * kk : 2 * kk + 2, :], sc[:], AF.Exp,
                bias=ln8_t[:], scale=SCALE,
            )

        for nq in range(2):
            probs_q = []

            def emit_head(h, nq=nq):
                probs_h = probsp.tile([P, MT, 512], F8, tag="probs")
                for kk in range(4):
                    scores_exp(h, probs_h, kk, nq)
                probs_q.append(probs_h)

            emit_head(0)
            emit_head(1)
            for m in range(8):  # head pairs (2m, 2m+1)
                h0, h1 = 2 * m, 2 * m + 1
                p_h0 = probs_q.pop(0)
                p_h1 = probs_q.pop(0)
                pav = pavp.tile([P, 512], F32, tag="pav")
                sps = auxp.tile([16, 512], F32, tag="aux")
                # even head: DR at base 0 (rows 0:64)
                for kk in range(4):
                    nc.tensor.matmul(
                        pav[0:HD, :],
                        v_sb[:, 2 * kk : 2 * kk + 2, h0 * HD : (h0 + 1) * HD],
                        p_h0[:, 2 * kk : 2 * kk + 2, :],
                        start=(kk == 0), stop=(kk == 3),
                        perf_mode=DR, skip_group_check=True,
                    )
                # S for even head (one-hot col 0)
                for kk in range(4):
                    nc.tensor.matmul(
                        sps[:], ohe_t[:],
                        p_h0[:, 2 * kk : 2 * kk + 2, :],
                        start=(kk == 0), stop=False,
                        perf_mode=DR, skip_group_check=True,
                    )
                if m < 7:
                    emit_head(2 * m + 2)
                # odd head: non-DR fp8 at base 64 (rows 64:128)
                for mk in range(MT):
                    nc.tensor.matmul(
                        pav[HD:P, :],
                        v_sb[:, mk, h1 * HD : (h1 + 1) * HD],
                        p_h1[:, mk, :],
                        start=(mk == 0), stop=(mk == MT - 1),
                        skip_group_check=True,
                    )
                # S for odd head (one-hot col 1) closes the pair group
                for kk in range(4):
                    nc.tensor.matmul(
                        sps[:], oho_t[:],
                        p_h1[:, 2 * kk : 2 * kk + 2, :],
                        start=False, stop=(kk == 3),
                        perf_mode=DR, skip_group_check=True,
                    )
                if m < 7:
                    emit_head(2 * m + 3)
                # drain S -> 1/S -> fp16 -> rb broadcast -> oT = pav * rb
                # (rows 2:15 of sps are zero; keep them out of the reciprocal
                # so no inf reaches the rb matmul)
                srow = srowp.tile([16, 512], F32, tag="srow")
                nc.vector.reciprocal(srow[0:2, :], sps[0:2, :])
                srow16 = srowp.tile([16, 512], F16, tag="srow16")
                nc.vector.tensor_copy(srow16[0:2, :], srow[0:2, :])
                rb = auxp.tile([P, 512], F32, tag="aux")
                nc.tensor.matmul(rb[:], sel_t[0:2, :, :], srow16[0:2, :],
                                 start=True, stop=True)
                rb16 = srowp.tile([P, 512], F16, tag="rb16")
                nc.vector.tensor_copy(rb16[:], rb[:])
                nc.vector.tensor_tensor(
                    out=oT[:, m, ts(nq, 512)], in0=pav[:], in1=rb16[:],
                    op=ALU.mult,
                )

        # ---- phase 4+5: proj + residual -> x1, LN2 fused per tile ----
        x1 = x1p.tile([P, MT, D], F16)
        h2hi = h2hip.tile([P, KC, N], F8, tag="h2hi")
        h2lo = h2lop.tile([P, KC, N], F8, tag="h2lo")
        for mt in range(MT):
            x_t = xload.tile([P, D], F32, tag="x_t")
            nc.sync.dma_start(x_t[:], x_d.ap()[ts(mt, P), :])
            ps = bigp.tile([P, N], F32, tag="big")
            for np_ in range(2):
                for kk in range(4):
                    nc.tensor.matmul(
                        ps[:, ts(np_, 512)],
                        oT[:, 2 * kk : 2 * kk + 2, ts(mt, P)],
                        proj_pieces[np_][:, 2 * kk : 2 * kk + 2, :],
                        start=(kk == 0), stop=(kk == 3),
                        perf_mode=DR, skip_group_check=True,
                    )
            nc.vector.scalar_tensor_tensor(
                x1[:, mt, :], ps[:], IWS, x_t[:], ALU.mult, ALU.add,
            )
            if apply_c1:
                nc.vector.tensor_add(x1[:, mt, :], x1[:, mt, :], c1row[:])
            layer_norm_to(x1[:, mt, :], h2hi, h2lo, ln2s, ln2b, mt, ln2_triv)

        # ---- phase 6: fc1 (3-pass fp8 DR) + gelu -> a1 fp8 ----
        # a1 [128, 32, 1024] fp8 reuses qT (chunks 0:16) + kT (16:32) slots
        a1a = qTp.tile([P, 16, N], F8, tag="qT")
        a1b = kTp.tile([P, 16, N], F8, tag="kT")

        for p8 in range(8):  # 512-wide hidden column pieces
            w1h_t = wpiece(w1hi_v, p8 * 512)
            w1l_t = wpiece(w1lo_v, p8 * 512)
            for nt in range(2):
                for mg in range(2):  # two mh chunks per psum
                    ps = bigp.tile([P, 2, 512], F32, tag="big")
                    for ml in range(2):
                        mh_l = mg * 2 + ml
                        for kk in range(4):
                            k2 = slice(2 * kk, 2 * kk + 2)
                            for wt, ht in ((w1h_t, h2hi), (w1l_t, h2hi),
                                           (w1h_t, h2lo)):
                                nc.tensor.matmul(
                                    ps[:, ml, :],
                                    wt[:, k2, ts(mh_l, P)],
                                    ht[:, k2, ts(nt, 512)],
                                    start=(kk == 0 and wt is w1h_t and ht is h2hi),
                                    stop=(kk == 3 and ht is h2lo),
                                    perf_mode=DR, skip_group_check=True,
                                )
                    mhg = p8 * 4 + mg * 2  # first of two mh chunks
                    a1_t = a1a if mhg < 16 else a1b
                    adst = a1_t[:, mhg % 16 : mhg % 16 + 2, ts(nt, 512)]
                    if fc1b_triv:
                        nc.scalar.activation(
                            adst, ps[:], AF.Gelu_apprx_tanh,
                            bias=bfc1[:, 0:1], scale=IWS,
                        )
                    else:
                        for ml in range(2):
                            nc.scalar.activation(
                                a1_t[:, mhg % 16 + ml, ts(nt, 512)],
                                ps[:, ml, :], AF.Gelu_apprx_tanh,
                                bias=bfc1[:, mhg + ml : mhg + ml + 1], scale=IWS,
                            )

        # ---- phase 7: fc2 (1-pass fp8 DR) + residual -> y ----
        # w2 fully resident (32KB fp8, streamed during fc1); one psum
        # accumulation over all 32 hid chunks per token tile -> one stt.
        # (w2 single fp8: measured rel_fro 1.67e-2 vs gate 2e-2.)
        w2h_t = w2pool.tile([P, 32, N], F8, tag="w2h")
        for c8 in range(4):
            nc.sync.dma_start(
                w2h_t[:, c8 * 8 : c8 * 8 + 8, :], w2hi_v[:, c8 * 8 : c8 * 8 + 8, :]
            )
        for mt in range(MT):
            ps = bigp.tile([P, N], F32, tag="big")
            for ncol in range(2):
                for g in range(2):
                    a1_t = a1a if g == 0 else a1b
                    for kk in range(8):
                        a2 = slice(2 * kk, 2 * kk + 2)
                        k2 = slice(g * 16 + 2 * kk, g * 16 + 2 * kk + 2)
                        nc.tensor.matmul(
                            ps[:, ts(ncol, 512)],
                            a1_t[:, a2, ts(mt, P)],
                            w2h_t[:, k2, ts(ncol, 512)],
                            start=(g == 0 and kk == 0), stop=(g == 1 and kk == 7),
                            perf_mode=DR, skip_group_check=True,
                        )
            y_sb = xload.tile([P, D], F16, tag="y_sb")
            nc.vector.scalar_tensor_tensor(
                y_sb[:], ps[:], IWS, x1[:, mt, :], ALU.mult, ALU.add,
            )
            if apply_bfc2:
                nc.vector.tensor_add(y_sb[:], y_sb[:], b2row[:])
            nc.sync.dma_start(y_d.ap()[ts(mt, P), :], y_sb[:])

    nc.compile()
    return nc


_cache = {}


def _get_nc(*key):
    if key not in _cache:
        _cache[key] = build_block(*key)
    return _cache[key]


def _host_consts():
    # rb matmul (non-DR): out[jm, n] = sum_k sel[k, j, m] * srow[k, n] with the
    # lhsT free index (j, m) flattened onto output partitions: partitions 0:64
    # (j=0, even head) read srow row 0; partitions 64:128 (j=1, odd) row 1.
    sel = np.zeros((16, 2, HD), np.float16)
    sel[0, 0, :] = 1.0
    sel[1, 1, :] = 1.0
    ohe = np.zeros((P, 2, 16), np.float32)
    ohe[:, :, 0] = 1.0
    oho = np.zeros((P, 2, 16), np.float32)
    oho[:, :, 1] = 1.0
    return sel, ohe.astype(E4), oho.astype(E4)


def kernel(
    x, w_qkv, b_qkv, w_proj, b_proj, ln1_scale, ln1_bias,
    ln2_scale, ln2_bias, w_fc1, b_fc1, w_fc2, b_fc2,
):
    x = np.asarray(x, np.float32)
    B = x.shape[0]
    b_qkv = np.asarray(b_qkv, np.float32)
    b_v = b_qkv[2 * D :]
    # exact fold: o includes +b_v after softmax-normalize (rows sum to 1),
    # so c1 = b_v @ w_proj + b_proj is a constant row added post-proj.
    c1 = b_v.astype(np.float64) @ np.asarray(w_proj, np.float64) + np.asarray(
        b_proj, np.float64
    )
    c1 = c1.astype(np.float32)
    bfc2 = np.asarray(b_fc2, np.float32)
    b_fc1 = np.asarray(b_fc1, np.float32)
    ln1_scale = np.asarray(ln1_scale, np.float32)
    ln1_bias = np.asarray(ln1_bias, np.float32)
    ln2_scale = np.asarray(ln2_scale, np.float32)
    ln2_bias = np.asarray(ln2_bias, np.float32)
    ln1_triv = bool(np.all(ln1_scale == 1) and np.all(ln1_bias == 0))
    ln2_triv = bool(np.all(ln2_scale == 1) and np.all(ln2_bias == 0))
    qk_triv = bool(np.all(b_qkv[: 2 * D] == 0))
    fc1b_triv = bool(np.all(b_fc1 == 0))
    apply_c1 = bool(np.any(c1 != 0))
    apply_bfc2 = bool(np.any(bfc2 != 0))

    nc = _get_nc(ln1_triv, ln2_triv, qk_triv, fc1b_triv, apply_c1, apply_bfc2)

    def q8(a):
        return np.ascontiguousarray(a).astype(E4)

    w_qkv8 = q8(np.asarray(w_qkv, np.float32) * WS)
    w_proj8 = q8(np.asarray(w_proj, np.float32) * WS)
    w1s = np.asarray(w_fc1, np.float32) * WS
    w1hi = q8(w1s)
    w1lo = q8(w1s - w1hi.astype(np.float32))
    w2s = np.asarray(w_fc2, np.float32) * WS
    w2hi = q8(w2s)
    sel, ohe, oho = _host_consts()

    base = {
        "w_qkv": w_qkv8,
        "w_proj": w_proj8,
        "w1hi": w1hi,
        "w1lo": w1lo,
        "w2hi": w2hi,
        "b_qkv": b_qkv,
        "b_fc1": b_fc1,
        "ln1_scale": ln1_scale,
        "ln1_bias": ln1_bias,
        "ln2_scale": ln2_scale,
        "ln2_bias": ln2_bias,
        "selc": sel,
        "ohe": ohe,
        "oho": oho,
    }
    if apply_c1:
        base["c1"] = c1
    if apply_bfc2:
        base["b_fc2c"] = bfc2

    in_maps = [dict(base, x=np.ascontiguousarray(x[i])) for i in range(B)]
    last_err = None
    for _attempt in range(3):
        try:
            res = run_bass_kernel_spmd(nc, in_maps, core_ids=list(range(B)))
            break
        except Exception as e:  # transient NRT/axon worker failures
            last_err = e
            import time as _time

            _time.sleep(2.0)
    else:
        raise last_err
    out = np.stack([res.results[i]["y"] for i in range(B)], axis=0)
    return np.ascontiguousarray(out.astype(np.float32))
